# revision 2
# baseline (speedup 1.0000x reference)
"""Trainium2 Bass kernel for an attention seq2seq decoder (nn_Decoder).

Transposed-layout design, v2.

Reference math (per batch row b):
  att_h = eout @ wW.T                      (wb folded into state bias)
  scan over L-1 steps t:
    x = [emb[y_t], ctx]; h,c = LSTM(x, h, c; att_Wih, att_Whh, att_b)
    state = h @ vW.T + (vb + wb)
    scores = sum_d w_att_v[d] * tanh(state[d] + att_h[d,t']) + mbias
    alpha = softmax(scores); ctx = alpha @ eout
  att_fea = [h*ym, ctx*ym]
  dec scan: dh_t = LSTM(att_fea_t; dec_*)
  logit = ([att_fea, dh] * ym) @ cls_W.T + cls_b

Distribution: data-parallel over batch B=64 across 8 cores (8 rows/core),
all parameters replicated; the timestep scans stay local per core.

Device design notes (per core, 8 local rows in 2 groups of 4):
 - Everything recurrent lives TRANSPOSED: [128 (d%128), dchunk, b].  Gate
   matmuls, state matmul, score matmuls and context matmuls all use the
   batch (4) or a single column as the PE moving dimension, with the large
   tensors (weights, tanh tiles, eout) as the stationary operand.
 - gates come out of PE as gatesT [128 gate-dim, 8 chunks, b]; ACT tanh on
   [128, 32] replaces the old [4, 1024] stream.  sigmoid = 0.5(1+tanh(z/2))
   via host-halved i/f/o rows; hidden stored as 2h, cell as c/2.
 - att_fea is stored as [2h * ym, ctx * ym]; all consumers of the h part
   (dec_Wih, cls_W first/last thirds, vW) are pre-halved on the host, so
   the stored 2h needs no extra scaling and doubles as the recurrent h.
 - the big per-step tanh(state + att_h) over [d, T] is split across
   engines: ACT computes native tanh with the state add fused into the
   activation bias; DVE computes it via the addition formula
   tanh(s+a) = (ta+ts)/(1+ta*ts) with ta = tanh(att_h) precomputed in f32
   and the reciprocal evaluated by a custom 8-stage DVE op (NOT-seed +
   one Newton step); the numerator ta+ts is produced on the idle GPSIMD
   (Pool) engine.
 - scores come out of PE transposed [128 (t%128), b, tc] so exp is one
   [128, 32] ACT op and the exp columns feed the ctx matmuls directly
   (no alpha transposes).  Softmax sums: DVE reduce over tc + an
   all-ones f32 matmul over the t partitions.
"""

import numpy as np
import ml_dtypes
from dataclasses import dataclass

import concourse.bass as bass
import concourse.bacc as bacc
import concourse.tile as tile
import concourse.mybir as mybir
from concourse.masks import make_identity

F32 = mybir.dt.float32
BF16 = mybir.dt.bfloat16
AF = mybir.ActivationFunctionType
OP = mybir.AluOpType
AX = mybir.AxisListType
BF = ml_dtypes.bfloat16

D = 256  # model dim (layout hardcodes D == 2*128)


# ---------------------------------------------------------------------------
# custom DVE op: out = (Src0 + s0) * approx(1/Src1), Src1 = den in (0, 2).
# NOT-seed: z = den * bitcast(~den) lands in [-4.5, -4]; a relative-minimax
# linear fit 1/den ~= m*(B*z + A) on that interval gives ~1.7e-3 rel err.
# ---------------------------------------------------------------------------

TANH_RECIP_S1 = -0.0554592   # B
TANH_RECIP_IMM2 = -0.4714030  # A


def _register_tanh_recip():
    import concourse.dve_ops as dve_ops_mod
    from concourse.dve_ops import DveOp
    from concourse.dve_spec import AluOp, Bin, C0, C1, C2, Spec, Src0, \
        Src1, _has_src1, lower
    from concourse.dve_uop import DveOpSpec

    name = "TANH_RECIP_APPLY_ANT"
    if name in dve_ops_mod.CUSTOM_DVE_SPECS:
        return next(op for op in dve_ops_mod.OPS if op.name == name)

    m = Bin(AluOp.BITWISE_NOT, Src1, Src1)
    z = Bin(AluOp.MULTIPLY, Src1, m)
    f = Bin(AluOp.ADD, Bin(AluOp.MULTIPLY, z, C1), C2)
    r = Bin(AluOp.MULTIPLY, m, f)
    body = Bin(AluOp.MULTIPLY, Bin(AluOp.ADD, Src0, C0), r)

    def ref(in0, in1, c0, c1, c2):
        c0 = np.float32(c0) if isinstance(c0, float) else c0.astype(np.float32)
        c1 = np.float32(c1) if isinstance(c1, float) else c1.astype(np.float32)
        x = in1.astype(np.float32)
        m_ = (~x.view(np.int32)).view(np.float32)
        r_ = m_ * (x * m_ * c1 + np.float32(c2))
        return (in0.astype(np.float32) + c0) * r_

    spec = Spec(body=body, reference=ref)
    row = 0x1E
    assert row not in dve_ops_mod._SUB_OPCODE_FOR_NAME.values()
    dve_ops_mod._SUB_OPCODE_FOR_NAME[name] = row
    shas = {}
    for ver in ("v3", "v4"):
        s = DveOpSpec(name=name, opcode=row, uops=lower(spec, ver=ver),
                      rd1_en=_has_src1(spec))
        shas[ver] = s.sha(ver)
    op = DveOp(name, spec, subdim=False, uops_sha=shas)
    dve_ops_mod.OPS.append(op)
    dve_ops_mod.CUSTOM_DVE_SPECS[name] = spec
    return op


# unit assignment: units are (dc, bb) per group; 3 per group go to the
# DVE addition-formula path, the rest to native ACT tanh.
DVE_UNITS = {
    0: ((0, 2), (1, 2), (1, 3)),
    1: ((0, 2), (1, 2), (0, 3)),
}
# den producer per dve-unit index within the group: engine round-robin
DEN_ENGINE = ("pool", "pool", "dve")


@dataclass(frozen=True)
class Cfg:
    T: int = 1024          # encoder length
    L: int = 65            # decoder length (steps = L-1)
    V: int = 4235          # vocab
    BL: int = 8            # batch rows per core
    num_devices: int = 8
    with_mbias: bool = False
    with_ymask: bool = False
    exp_shift: float = 0.0   # constant subtracted inside exp (softmax-invariant)

    @property
    def NS(self):
        return self.L - 1

    @property
    def NT(self):
        return self.NS * self.BL  # total (t,g,b) rows

    @property
    def TC(self):
        return self.T // 128


def build_program(cfg: Cfg):
    NS, NT, T, V, TC = cfg.NS, cfg.NT, cfg.T, cfg.V, cfg.TC
    BL = cfg.BL
    assert BL == 8
    assert T % 128 == 0 and NT % 128 == 0
    NTC = NT // 128               # row chunks of pregates (4)
    MC = NT // 128                # classifier row chunks (4)
    NV = (V + 511) // 512         # vocab chunks (9)

    OP_T = _register_tanh_recip()

    # dve slot ids for ta_sb
    dve_slot = {}
    for g in (0, 1):
        for u in DVE_UNITS[g]:
            dve_slot[(g,) + u] = len(dve_slot)
    NDVE = len(dve_slot)

    nc = bacc.Bacc("TRN2", target_bir_lowering=False, debug=False,
                   num_devices=cfg.num_devices)

    def din(name, shape, dt=BF16):
        return nc.dram_tensor(name, shape, dt, kind="ExternalInput").ap()

    eout_d = din("eout_r", [128, BL, TC, D])
    embr_d = din("embr", [128, NTC, D])
    wihe_d = din("wihe", [128, 2, 1024])
    wihc_d = din("wihc", [128, 2, 1024])
    whh_d = din("whh", [128, 2, 1024])
    attb_d = din("attb", [1, 1024])
    ww_d = din("ww", [128, 2, 2, 128])
    vw_d = din("vw", [128, 2, 2, 128])
    biasvw_d = din("biasvw", [128, 2], F32)
    wv_d = din("wv", [128, 2])
    dwih_d = din("dwih", [128, 4, 1024])
    dwhh_d = din("dwhh", [128, 2, 1024])
    decb_d = din("decb", [1, 1024])
    cls_d = din("cls", [128, 6, V])
    clsb_d = din("clsb", [1, V])
    if cfg.with_ymask:
        ymT_d = din("ymT", [128, NS, BL], F32)
    if cfg.with_mbias:
        mbiasT_d = din("mbiasT", [128, BL, TC], F32)
    out_d = nc.dram_tensor("logits", [MC, 128, V], F32,
                           kind="ExternalOutput").ap()

    with tile.TileContext(nc) as tc:
        import contextlib
        stack = contextlib.ExitStack()
        with stack:
            singles = stack.enter_context(tc.tile_pool(name="singles", bufs=1))

            # ---------- persistent SBUF ----------
            eout_sb = singles.tile([128, BL, TC, D], BF16)
            atth_sb = singles.tile([128, 2, BL, T], BF16)
            if NDVE:
                ta_sb = singles.tile([128, NDVE, T], F32)
            pregates_sb = singles.tile([128, NTC, 1024], BF16)
            affT_sb = singles.tile([128, 4, NT], BF16)
            dhT_sb = singles.tile([128, 2, NT], BF16)
            wihc_sb = singles.tile([128, 2, 1024], BF16)
            whh_sb = singles.tile([128, 2, 1024], BF16)
            attb_sb = singles.tile([1, 1024], BF16)
            vw_sb = singles.tile([128, 2, 2, 128], BF16)
            biasvw_sb = singles.tile([128, 2], F32)
            wv_sb = singles.tile([128, 2], BF16)
            dwih_sb = singles.tile([128, 4, 1024], BF16)
            dwhh_sb = singles.tile([128, 2, 1024], BF16)
            decb_sb = singles.tile([1, 1024], BF16)
            clsb_sb = singles.tile([1, V], BF16)
            ident = singles.tile([128, 128], BF16)
            ones_sb = singles.tile([1, 128], BF16)
            onesf_sb = singles.tile([128, 128], F32)

            state_sb = singles.tile([128, 2, BL], F32)
            ts_sb = singles.tile([128, 2, BL], F32)
            cT_sb = singles.tile([128, 2, BL], F32)      # att c/2, cols g4+bb
            cdT_sb = singles.tile([128, 2, BL], F32)     # dec c/2
            if cfg.with_ymask:
                hT_sb = singles.tile([128, 2, BL], BF16)     # recurrent 2h
                ctxT_sb = singles.tile([128, 2, BL], BF16)   # recurrent ctx
                hdT_sb = singles.tile([128, 2, BL], BF16)    # dec recurrent 2h
                ymT_sb = singles.tile([128, NS, BL], F32)
            if cfg.with_mbias:
                mbiasT_sb = singles.tile([128, BL, TC], F32)

            # ---------- input DMAs ----------
            for dst, src in [
                (wihc_sb, wihc_d), (whh_sb, whh_d), (attb_sb, attb_d),
                (vw_sb, vw_d), (biasvw_sb, biasvw_d), (wv_sb, wv_d),
                (dwih_sb, dwih_d), (dwhh_sb, dwhh_d),
                (decb_sb, decb_d), (clsb_sb, clsb_d),
            ]:
                nc.sync.dma_start(out=dst[:], in_=src)
            if cfg.with_ymask:
                nc.sync.dma_start(out=ymT_sb[:], in_=ymT_d)
            if cfg.with_mbias:
                nc.sync.dma_start(out=mbiasT_sb[:], in_=mbiasT_d)
            for b_ in range(BL):
                nc.sync.dma_start(out=eout_sb[:, b_, :, :],
                                  in_=eout_d[:, b_, :, :])

            make_identity(nc, ident[:])
            nc.vector.memset(ones_sb[:], 1.0)
            nc.vector.memset(onesf_sb[:], 1.0)
            nc.vector.memset(cT_sb[:], 0.0)
            nc.vector.memset(cdT_sb[:], 0.0)
            nc.vector.memset(state_sb[:], 0.0)
            nc.vector.memset(ts_sb[:], 0.0)
            if cfg.with_ymask:
                nc.vector.memset(hT_sb[:], 0.0)
                nc.vector.memset(ctxT_sb[:], 0.0)
                nc.vector.memset(hdT_sb[:], 0.0)

            # ---------- prep phase ----------
            with tc.tile_pool(name="prep_ps", bufs=3, space="PSUM") as pps, \
                 tc.tile_pool(name="prep_ps2", bufs=3, space="PSUM") as pps2, \
                 tc.tile_pool(name="prep_sb", bufs=2) as psb:
                embr_sb = psb.tile([128, NTC, D], BF16, bufs=1)
                embT_sb = psb.tile([128, 2, NT], BF16, bufs=1)
                wihe_sb = psb.tile([128, 2, 1024], BF16, bufs=1)
                ww_sb = psb.tile([128, 2, 2, 128], BF16, bufs=1)
                nc.sync.dma_start(out=embr_sb[:], in_=embr_d)
                nc.sync.dma_start(out=wihe_sb[:], in_=wihe_d)
                nc.sync.dma_start(out=ww_sb[:], in_=ww_d)
                # embT: transpose embr row-chunks -> [d, row]
                for m in range(NTC):
                    for ch in range(2):
                        tp = pps.tile([128, 128], BF16, tag="tp")
                        nc.tensor.transpose(
                            tp[:], embr_sb[:, m, ch * 128:(ch + 1) * 128],
                            ident[:])
                        nc.vector.tensor_copy(
                            embT_sb[:, ch, m * 128:(m + 1) * 128], tp[:])
                # att pregates = embed @ WihE.T + att_b  -> [row, 1024]
                for m in range(NTC):
                    for half in range(2):
                        hs = slice(half * 512, (half + 1) * 512)
                        gp = pps2.tile([128, 512], F32, tag="gp")
                        nc.tensor.matmul(gp[:], ones_sb[0:1, :],
                                         attb_sb[0:1, hs],
                                         start=True, stop=False)
                        for kc in range(2):
                            nc.tensor.matmul(
                                gp[:],
                                embT_sb[:, kc, m * 128:(m + 1) * 128],
                                wihe_sb[:, kc, hs],
                                start=False, stop=(kc == 1))
                        nc.vector.tensor_copy(pregates_sb[:, m, hs], gp[:])
                # att_h transposed [d, t] per b: eoutT then ww @ eoutT
                for b in range(BL):
                    g, bb = divmod(b, 4)
                    eoutT_b = psb.tile([128, 2, T], BF16, tag="eoutT")
                    for t_c in range(TC):
                        for ch in range(2):
                            tp2 = pps.tile([128, 128], BF16, tag="tp")
                            nc.tensor.transpose(
                                tp2[:],
                                eout_sb[:, b, t_c, ch * 128:(ch + 1) * 128],
                                ident[:])
                            dst = eoutT_b[:, ch, t_c * 128:(t_c + 1) * 128]
                            if (t_c * 2 + ch) % 2 == 0:
                                nc.vector.tensor_copy(dst, tp2[:])
                            else:
                                nc.scalar.copy(dst, tp2[:])
                    for dc in range(2):
                        is_dve = (g, dc, bb) in dve_slot
                        for n in range(T // 512):
                            ap = pps2.tile([128, 512], F32, tag="gp")
                            for kc in range(2):
                                nc.tensor.matmul(
                                    ap[:],
                                    ww_sb[:, kc, dc, :],
                                    eoutT_b[:, kc, n * 512:(n + 1) * 512],
                                    start=(kc == 0), stop=(kc == 1))
                            ns = slice(n * 512, (n + 1) * 512)
                            bvw = biasvw_sb[:, dc:dc + 1]
                            if is_dve:
                                # ta = tanh(att_h + vb + wb), f32
                                sl = dve_slot[(g, dc, bb)]
                                nc.scalar.activation(ta_sb[:, sl, ns], ap[:],
                                                     AF.Tanh, bias=bvw)
                            else:
                                # atth' = att_h + vb + wb, bf16
                                nc.vector.tensor_scalar(
                                    atth_sb[:, dc, b, ns], ap[:], bvw, None,
                                    OP.add)

            # ---------- scan phase ----------
            with tc.tile_pool(name="psG", bufs=2, space="PSUM") as psG, \
                 tc.tile_pool(name="psS", bufs=2, space="PSUM") as psS, \
                 tc.tile_pool(name="psM", bufs=2, space="PSUM") as psM, \
                 tc.tile_pool(name="psL", bufs=2, space="PSUM") as psL, \
                 tc.tile_pool(name="scsb", bufs=3) as scsb, \
                 tc.tile_pool(name="thp", bufs=4) as thp, \
                 tc.tile_pool(name="nump", bufs=3) as nump, \
                 tc.tile_pool(name="cwp", bufs=2) as cwp, \
                 tc.tile_pool(name="csb", bufs=2) as csb:

                def rc_prev(g, t):
                    return (t - 1) * 8 + g * 4

                def att_gates(g, t):
                    """gatesT [128, 8, 4] for group g step t (PE)."""
                    r0 = t * 8 + g * 4
                    m, p0 = divmod(r0, 128)
                    rp = rc_prev(g, t)
                    gp = psG.tile([128, 8, 4], F32, tag="g")
                    for gc in range(8):
                        gs = slice(gc * 128, (gc + 1) * 128)
                        first = (t == 0 and not cfg.with_ymask)
                        nc.tensor.matmul(gp[:, gc, :],
                                         pregates_sb[:, m, gs],
                                         ident[:, p0:p0 + 4],
                                         start=True, stop=first)
                        if first:
                            continue
                        if cfg.with_ymask:
                            ctx_rhs = [ctxT_sb[:, kc, g * 4:g * 4 + 4]
                                       for kc in range(2)]
                            h_rhs = [hT_sb[:, kc, g * 4:g * 4 + 4]
                                     for kc in range(2)]
                        else:
                            ctx_rhs = [affT_sb[:, 2 + kc, rp:rp + 4]
                                       for kc in range(2)]
                            h_rhs = [affT_sb[:, kc, rp:rp + 4]
                                     for kc in range(2)]
                        for kc in range(2):
                            nc.tensor.matmul(gp[:, gc, :],
                                             wihc_sb[:, kc, gs], ctx_rhs[kc],
                                             start=False, stop=False)
                        for kc in range(2):
                            nc.tensor.matmul(gp[:, gc, :],
                                             whh_sb[:, kc, gs], h_rhs[kc],
                                             start=False, stop=(kc == 1))
                    return gp

                def lstm_tail1(g, t, gp):
                    """tanh gates + c/h update for group g step t."""
                    g4 = g * 4
                    r0 = t * 8 + g * 4
                    tg = scsb.tile([128, 8, 4], BF16, tag="tg")
                    nc.scalar.activation(tg[:], gp[:], AF.Tanh)
                    ti = tg[:, 0:2, :]
                    tf = tg[:, 2:4, :]
                    tgg = tg[:, 4:6, :]
                    to = tg[:, 6:8, :]
                    cc = cT_sb[:, :, g4:g4 + 4]
                    aT = scsb.tile([128, 2, 4], F32, tag="aT")
                    bT = scsb.tile([128, 2, 4], F32, tag="bT")
                    tT = scsb.tile([128, 2, 4], F32, tag="tT")
                    nc.vector.scalar_tensor_tensor(aT[:], tf, 1.0, cc,
                                                   OP.add, OP.mult)
                    nc.vector.scalar_tensor_tensor(bT[:], ti, 1.0, tgg,
                                                   OP.add, OP.mult)
                    nc.vector.scalar_tensor_tensor(tT[:], bT[:], 0.5, aT[:],
                                                   OP.mult, OP.add)
                    tcb = scsb.tile([128, 2, 4], BF16, tag="tcb")
                    nc.scalar.activation(tcb[:], tT[:], AF.Tanh)
                    # hH = 2h = (to+1)*tanh(c)
                    if cfg.with_ymask:
                        hh = hT_sb[:, :, g4:g4 + 4]
                        nc.vector.scalar_tensor_tensor(hh, to, 1.0, tcb[:],
                                                       OP.add, OP.mult)
                        ym = ymT_sb[:, t, g4:g4 + 4]
                        ymb = bass.AP(tensor=ym.tensor, offset=ym.offset,
                                      ap=[ym.ap[0], [0, 2], [1, 4]])
                        nc.vector.tensor_tensor(affT_sb[:, 0:2, r0:r0 + 4],
                                                hh, ymb, OP.mult)
                    else:
                        nc.vector.scalar_tensor_tensor(
                            affT_sb[:, 0:2, r0:r0 + 4], to, 1.0, tcb[:],
                            OP.add, OP.mult)
                    nc.vector.tensor_scalar_mul(cc, tT[:], 0.5)

                def lstm_tail2(g, t):
                    """state = vw @ hH (bias lives in atth/ta); ts = tanh."""
                    g4 = g * 4
                    r0 = t * 8 + g * 4
                    stp = psM.tile([128, 24], F32, tag="m")
                    for mc2 in range(2):
                        for kc in range(2):
                            h_rhs = (hT_sb[:, kc, g4:g4 + 4] if cfg.with_ymask
                                     else affT_sb[:, kc, r0:r0 + 4])
                            nc.tensor.matmul(
                                stp[:, mc2 * 4:mc2 * 4 + 4],
                                vw_sb[:, kc, mc2, :], h_rhs,
                                start=(kc == 0), stop=(kc == 1))
                    stv = bass.AP(tensor=stp.tensor, offset=stp.offset,
                                  ap=[stp.ap[0], [4, 2], [1, 4]])
                    nc.scalar.activation(ts_sb[:, :, g4:g4 + 4], stv, AF.Tanh)
                    return stv

                def unit_act(g, dc, bb, t):
                    c = g * 4 + bb
                    th = thp.tile([128, T], BF16, tag="th", bufs=10)
                    nc.scalar.activation(th[:], atth_sb[:, dc, c, :],
                                         AF.Tanh,
                                         bias=state_sb[:, dc, c:c + 1])
                    return th

                def scores_b(sc, bb, th0, th1):
                    for t_c in range(TC):
                        nc.tensor.matmul(sc[:, bb, t_c:t_c + 1],
                                         th0[:, t_c * 128:(t_c + 1) * 128],
                                         wv_sb[:, 0:1], start=True, stop=False)
                        nc.tensor.matmul(sc[:, bb, t_c:t_c + 1],
                                         th1[:, t_c * 128:(t_c + 1) * 128],
                                         wv_sb[:, 1:2], start=False, stop=True)

                def emit_dens(g, t, stv):
                    """dens on Pool/DVE right after ts; state psum copy."""
                    g4 = g * 4
                    dve = DVE_UNITS[g]
                    dens = {}
                    for k, (dc, bb) in enumerate(dve):
                        c = g4 + bb
                        sl = dve_slot[(g, dc, bb)]
                        den = nump.tile([128, T], F32, tag="nm", bufs=5)
                        eng = (nc.gpsimd if DEN_ENGINE[k] == "pool"
                               else nc.vector)
                        eng.tensor_scalar(den[:], ta_sb[:, sl, :],
                                          ts_sb[:, dc, c:c + 1], 1.0,
                                          OP.mult, OP.add)
                        dens[(dc, bb)] = den
                    # state for the ACT units' fused bias (plain psum copy)
                    nc.vector.tensor_copy(state_sb[:, :, g4:g4 + 4], stv)
                    return dens

                def big_tanh(g, t, dens):
                    """units + score matmuls; returns the sc psum tile."""
                    g4 = g * 4
                    sc = psS.tile([128, 4, TC], F32, tag="sc")
                    dve = DVE_UNITS[g]
                    ths = {}
                    for (dc, bb) in dve:
                        c = g4 + bb
                        sl = dve_slot[(g, dc, bb)]
                        th = thp.tile([128, T], BF16, tag="th", bufs=10)
                        nc.vector._custom_dve(
                            OP_T, out=th[:], in0=ta_sb[:, sl, :],
                            in1=dens[(dc, bb)][:],
                            s0=ts_sb[:, dc, c:c + 1],
                            s1=TANH_RECIP_S1, imm2=TANH_RECIP_IMM2)
                        ths[(dc, bb)] = th
                    order = sorted(
                        [(bb, dc) for dc in range(2) for bb in range(4)],
                        key=lambda p: ((p[1], p[0]) in dve, p[0]))
                    done = set()
                    for bb, dc in order:
                        if (dc, bb) not in ths:
                            ths[(dc, bb)] = unit_act(g, dc, bb, t)
                        done.add((dc, bb))
                        if (0, bb) in done and (1, bb) in done:
                            scores_b(sc, bb, ths[(0, bb)], ths[(1, bb)])
                    return sc

                def finish_a(g, t, sc):
                    """exp + per-partition partial softmax sums."""
                    g4 = g * 4
                    exp_sb = scsb.tile([128, 4, TC], BF16, tag="exp")
                    if cfg.with_mbias:
                        scm = scsb.tile([128, 4, TC], F32, tag="scm")
                        nc.vector.tensor_tensor(scm[:], sc[:],
                                                mbiasT_sb[:, g4:g4 + 4, :],
                                                OP.add)
                        nc.scalar.activation(exp_sb[:], scm[:], AF.Exp,
                                             bias=float(-cfg.exp_shift))
                    else:
                        nc.scalar.activation(exp_sb[:], sc[:], AF.Exp,
                                             bias=float(-cfg.exp_shift))
                    sp = scsb.tile([128, 4], F32, tag="sp")
                    nc.vector.tensor_reduce(sp[:], exp_sb[:], AX.X, OP.add)
                    return exp_sb, sp

                def finish_b(g, t, exp_sb, sp):
                    """ctx matmuls, softmax total, ctx store."""
                    g4 = g * 4
                    r0 = t * 8 + g * 4
                    combo = psM.tile([128, 24], F32, tag="m")
                    for dch in range(2):
                        for bb in range(4):
                            col = combo[:, dch * 4 + bb:dch * 4 + bb + 1]
                            for t_c in range(TC):
                                nc.tensor.matmul(
                                    col,
                                    eout_sb[:, g4 + bb, t_c,
                                            dch * 128:(dch + 1) * 128],
                                    exp_sb[:, bb, t_c:t_c + 1],
                                    start=(t_c == 0), stop=(t_c == TC - 1))
                    es = combo[:, 16:20]
                    nc.tensor.matmul(es, onesf_sb[:], sp[:],
                                     start=True, stop=True)
                    rcp = scsb.tile([128, 4], F32, tag="rcp")
                    nc.vector.reciprocal(rcp[:], es)
                    ctxv = bass.AP(tensor=combo.tensor, offset=combo.offset,
                                   ap=[combo.ap[0], [4, 2], [1, 4]])
                    rcb = bass.AP(tensor=rcp.tensor, offset=rcp.offset,
                                  ap=[rcp.ap[0], [0, 2], [1, 4]])
                    if cfg.with_ymask:
                        ct = ctxT_sb[:, :, g4:g4 + 4]
                        nc.vector.tensor_tensor(ct, ctxv, rcb, OP.mult)
                        ym = ymT_sb[:, t, g4:g4 + 4]
                        ymb = bass.AP(tensor=ym.tensor, offset=ym.offset,
                                      ap=[ym.ap[0], [0, 2], [1, 4]])
                        nc.vector.tensor_tensor(affT_sb[:, 2:4, r0:r0 + 4],
                                                ct, ymb, OP.mult)
                    else:
                        nc.vector.tensor_tensor(affT_sb[:, 2:4, r0:r0 + 4],
                                                ctxv, rcb, OP.mult)

                def dec_step(u):
                    """dec LSTM step u over all 8 rows."""
                    r0 = u * 8
                    rp = r0 - 8
                    gp = psG.tile([128, 8, 8], F32, tag="g")
                    skip_h = (u == 0 and not cfg.with_ymask)
                    for gc in range(8):
                        gs = slice(gc * 128, (gc + 1) * 128)
                        nc.tensor.matmul(gp[:, gc, :],
                                         decb_sb[0:1, gs], ones_sb[0:1, 0:8],
                                         start=True, stop=False)
                        for ch in range(4):
                            nc.tensor.matmul(gp[:, gc, :],
                                             dwih_sb[:, ch, gs],
                                             affT_sb[:, ch, r0:r0 + 8],
                                             start=False,
                                             stop=(skip_h and ch == 3))
                        if skip_h:
                            continue
                        for kc in range(2):
                            h_rhs = (hdT_sb[:, kc, :] if cfg.with_ymask
                                     else dhT_sb[:, kc, rp:rp + 8])
                            nc.tensor.matmul(gp[:, gc, :],
                                             dwhh_sb[:, kc, gs], h_rhs,
                                             start=False, stop=(kc == 1))
                    tg = scsb.tile([128, 8, 8], BF16, tag="dtg")
                    nc.scalar.activation(tg[:], gp[:], AF.Tanh)
                    ti = tg[:, 0:2, :]
                    tf = tg[:, 2:4, :]
                    tgg = tg[:, 4:6, :]
                    to = tg[:, 6:8, :]
                    aT = scsb.tile([128, 2, 8], F32, tag="daT")
                    bT = scsb.tile([128, 2, 8], F32, tag="dbT")
                    tT = scsb.tile([128, 2, 8], F32, tag="dtT")
                    nc.vector.scalar_tensor_tensor(aT[:], tf, 1.0, cdT_sb[:],
                                                   OP.add, OP.mult)
                    nc.vector.scalar_tensor_tensor(bT[:], ti, 1.0, tgg,
                                                   OP.add, OP.mult)
                    nc.vector.scalar_tensor_tensor(tT[:], bT[:], 0.5, aT[:],
                                                   OP.mult, OP.add)
                    tcb = scsb.tile([128, 2, 8], BF16, tag="dtcb")
                    nc.scalar.activation(tcb[:], tT[:], AF.Tanh)
                    if cfg.with_ymask:
                        nc.vector.scalar_tensor_tensor(hdT_sb[:], to, 1.0,
                                                       tcb[:], OP.add, OP.mult)
                        ym = ymT_sb[:, u, :]
                        ymb = bass.AP(tensor=ym.tensor, offset=ym.offset,
                                      ap=[ym.ap[0], [0, 2], [1, 8]])
                        nc.vector.tensor_tensor(dhT_sb[:, :, r0:r0 + 8],
                                                hdT_sb[:], ymb, OP.mult)
                    else:
                        nc.vector.scalar_tensor_tensor(
                            dhT_sb[:, :, r0:r0 + 8], to, 1.0, tcb[:],
                            OP.add, OP.mult)
                    nc.vector.tensor_scalar_mul(cdT_sb[:], tT[:], 0.5)

                def cls_dma(k):
                    """prefetch classifier weights for unit k (one DMA)."""
                    nv = k % NV
                    nn = min(512, V - nv * 512)
                    ns = slice(nv * 512, nv * 512 + nn)
                    wt = cwp.tile([128, 6, 512], BF16, tag="wt")
                    nc.sync.dma_start(out=wt[:, :, 0:nn], in_=cls_d[:, :, ns])
                    return wt

                cls_out = []   # (k, lp) awaiting copy-out

                def cls_flush():
                    while cls_out:
                        k, lp = cls_out.pop(0)
                        m, nv = divmod(k, NV)
                        nn = min(512, V - nv * 512)
                        ns = slice(nv * 512, nv * 512 + nn)
                        lsb = csb.tile([128, 512], F32, tag="lsb")
                        nc.vector.tensor_copy(lsb[:, 0:nn], lp[:, 0:nn])
                        nc.sync.dma_start(out=out_d[m, :, ns],
                                          in_=lsb[:, 0:nn])

                def cls_mm(k, wt):
                    """classifier matmuls for unit k = m*NV + nv."""
                    m, nv = divmod(k, NV)
                    ms = slice(m * 128, (m + 1) * 128)
                    nn = min(512, V - nv * 512)
                    ns = slice(nv * 512, nv * 512 + nn)
                    cls_flush()   # previous unit's psum is long done
                    lp = psL.tile([128, 512], F32, tag="lp")
                    nc.tensor.matmul(lp[:, 0:nn], ones_sb[0:1, :],
                                     clsb_sb[0:1, ns],
                                     start=True, stop=False)
                    for ch in range(4):
                        nc.tensor.matmul(lp[:, 0:nn], affT_sb[:, ch, ms],
                                         wt[:, ch, 0:nn],
                                         start=False, stop=False)
                    for ch in range(2):
                        nc.tensor.matmul(lp[:, 0:nn], dhT_sb[:, ch, ms],
                                         wt[:, 4 + ch, 0:nn],
                                         start=False, stop=(ch == 1))
                    cls_out.append((k, lp))

                # ---------------- main loop ----------------
                pend = {}
                dec_done = 0
                cls_done = 0
                cls_pre = 0
                cls_q = []

                def cls_ready(k):
                    m = k // NV
                    # rows m*128..(m+1)*128 need dec steps up to 16(m+1)
                    return m < MC - 1 and dec_done >= 16 * (m + 1)

                pend_gates = {}
                for t in range(NS):
                    for g in (0, 1):
                        og = 1 - g
                        # PE order per half: [state] [ctx og + esum] [hoisted
                        # og gates for its next step] [unit scores] [dec] [cls]
                        gp = pend_gates.pop(g, None)
                        if gp is None:
                            gp = att_gates(g, t)
                        fo = None
                        if og in pend:
                            pt, psc = pend.pop(og)
                            fo = (pt, *finish_a(og, pt, psc))
                        lstm_tail1(g, t, gp)
                        stv = lstm_tail2(g, t)
                        if fo is not None:
                            finish_b(og, *fo)
                            nt = fo[0] + 1
                            if nt < NS:
                                pend_gates[og] = att_gates(og, nt)
                        pend[g] = (t, big_tanh(g, t, emit_dens(g, t, stv)))
                        if dec_done < t:
                            dec_step(dec_done)
                            dec_done += 1
                        # classifier: prefetch weights ahead, matmul when
                        # the rows' dec outputs are complete
                        if (cls_pre < MC * NV and cls_pre - cls_done < 2
                                and cls_ready(max(cls_pre - 1, 0))):
                            cls_q.append(cls_dma(cls_pre))
                            cls_pre += 1
                        if cls_q and cls_ready(cls_done):
                            cls_mm(cls_done, cls_q.pop(0))
                            cls_done += 1
                for g in (0, 1):
                    if g in pend:
                        pt, psc = pend.pop(g)
                        finish_b(g, pt, *finish_a(g, pt, psc))
                while dec_done < NS:
                    dec_step(dec_done)
                    dec_done += 1
                while cls_done < MC * NV:
                    while cls_pre < MC * NV and cls_pre - cls_done < 2:
                        cls_q.append(cls_dma(cls_pre))
                        cls_pre += 1
                    cls_mm(cls_done, cls_q.pop(0))
                    cls_done += 1
                cls_flush()

    nc.compile()
    return nc


# ---------------------------------------------------------------------------
# host marshaling
# ---------------------------------------------------------------------------

def host_prep_shared(cfg: Cfg, emb, att_Wih, att_Whh, att_b, wW, wb, vW, vb,
                     w_att_v, dec_Wih, dec_Whh, dec_b, cls_W, cls_b):
    """Weight preprocessing shared by all cores."""
    f = np.float32
    att_Wih = np.asarray(att_Wih, f).copy()
    att_Whh = np.asarray(att_Whh, f).copy()
    att_b = np.asarray(att_b, f).copy()
    dec_Wih = np.asarray(dec_Wih, f).copy()
    dec_Whh = np.asarray(dec_Whh, f).copy()
    dec_b = np.asarray(dec_b, f).copy()
    cls_W = np.asarray(cls_W, f).copy()
    # sigmoid(z) = 0.5*(1+tanh(z/2)): halve i,f,o rows (gate order i,f,g,o)
    ifo = np.r_[0:512, 768:1024]
    for W in (att_Wih, dec_Wih, att_Whh, dec_Whh):
        W[ifo] *= 0.5
    for bvec in (att_b, dec_b):
        bvec[ifo] *= 0.5
    # hidden state stored as 2h: halve all 2h-consuming weights
    att_Whh *= 0.5
    dec_Whh *= 0.5
    vW05 = np.asarray(vW, f) * 0.5
    dec_Wih[:, 0:256] *= 0.5       # att_fea h part stored as 2h*ym
    cls_W[:, 0:256] *= 0.5         # idem
    cls_W[:, 512:768] *= 0.5       # dec h stored as 2h*ym

    def pack_kn(WT, kc):  # [K, N] -> [128, kc, N]
        K, N = WT.shape
        assert K == kc * 128
        return np.ascontiguousarray(
            WT.reshape(kc, 128, N).transpose(1, 0, 2)).astype(BF)

    wihe = pack_kn(att_Wih[:, 0:256].T, 2)
    wihc = pack_kn(att_Wih[:, 256:512].T, 2)
    whh = pack_kn(att_Whh.T, 2)
    dwih = pack_kn(dec_Wih.T, 4)
    dwhh = pack_kn(dec_Whh.T, 2)

    def pack_kmn(WT):  # [256, 256] -> [128, kc2, mc2, 128]
        return np.ascontiguousarray(
            WT.reshape(2, 128, 2, 128).transpose(1, 0, 2, 3)).astype(BF)

    vw = pack_kmn(vW05.T)
    ww = pack_kmn(np.asarray(wW, f).T)
    biasvw = np.ascontiguousarray(
        (np.asarray(vb, f) + np.asarray(wb, f)).reshape(2, 128).T)
    wv = np.ascontiguousarray(
        np.asarray(w_att_v, f).reshape(2, 128).T).astype(BF)   # [128, dc]
    cls = np.ascontiguousarray(
        cls_W.T.reshape(6, 128, cfg.V).transpose(1, 0, 2)).astype(BF)
    shared = dict(
        wihe=wihe, wihc=wihc, whh=whh,
        attb=att_b.reshape(1, 1024).astype(BF),
        vw=vw, ww=ww, biasvw=biasvw.astype(f), wv=wv,
        dwih=dwih, dwhh=dwhh, decb=dec_b.reshape(1, 1024).astype(BF),
        cls=cls, clsb=np.asarray(cls_b, f).reshape(1, cfg.V).astype(BF),
    )
    return shared


def host_prep_core(cfg: Cfg, c, eout, x_mask, y, y_mask, emb, shared):
    """Per-core input shards. b rows c*BL .. c*BL+BL."""
    f = np.float32
    BL, T, NS, TC, NT = cfg.BL, cfg.T, cfg.NS, cfg.TC, cfg.NT
    NTC = NT // 128
    sl = slice(c * BL, (c + 1) * BL)
    e = np.asarray(eout[sl], f)                       # [BL, T, D]
    eout_r = np.ascontiguousarray(
        e.reshape(BL, TC, 128, D).transpose(2, 0, 1, 3)).astype(BF)
    yv = np.asarray(y[sl])                            # [BL, L]
    embed = np.asarray(emb, f)[yv[:, :-1]]            # [BL, NS, D]
    embed_r = np.ascontiguousarray(
        embed.transpose(1, 0, 2).reshape(NT, D))      # [(t,b), D]
    embr = np.ascontiguousarray(
        embed_r.reshape(NTC, 128, D).transpose(1, 0, 2)).astype(BF)
    d = dict(shared)
    d.update(eout_r=eout_r, embr=embr)
    if cfg.with_ymask:
        ym = np.asarray(y_mask[sl], f)[:, 1:]         # [BL, NS]
        ymT = np.broadcast_to(ym.T[None], (128, NS, BL))
        d["ymT"] = np.ascontiguousarray(ymT).astype(f)
    if cfg.with_mbias:
        mb = (np.asarray(x_mask[sl], f)[..., 0] - 1.0) * 1e30  # [BL, T]
        mbT = mb.T.reshape(TC, 128, BL).transpose(1, 2, 0)     # [128, BL, TC]
        d["mbiasT"] = np.ascontiguousarray(mbT).astype(f)
    return d


def host_post(cfg: Cfg, outs):
    """Reassemble [MC,128,V] per-core row-major (t,b) results -> [B, NS, V]."""
    parts = []
    for o in outs:
        lg = o.reshape(cfg.NT, cfg.V).reshape(cfg.NS, cfg.BL, cfg.V)
        parts.append(np.ascontiguousarray(lg.transpose(1, 0, 2)))
    return np.concatenate(parts, axis=0)


_PROG_CACHE = {}


def _get_program(cfg: Cfg):
    if cfg not in _PROG_CACHE:
        _PROG_CACHE[cfg] = build_program(cfg)
    return _PROG_CACHE[cfg]


def run(cfg: Cfg, inputs, trace=False):
    from concourse.bass_utils import run_bass_kernel_spmd
    nc = _get_program(cfg)
    shared = host_prep_shared(
        cfg, inputs["emb"], inputs["att_Wih"], inputs["att_Whh"],
        inputs["att_b"], inputs["wW"], inputs["wb"], inputs["vW"],
        inputs["vb"], inputs["w_att_v"], inputs["dec_Wih"],
        inputs["dec_Whh"], inputs["dec_b"], inputs["cls_W"], inputs["cls_b"])
    in_maps = [
        host_prep_core(cfg, c, inputs["eout"], inputs["x_mask"], inputs["y"],
                       inputs["y_mask"], inputs["emb"], shared)
        for c in range(cfg.num_devices)
    ]
    res = run_bass_kernel_spmd(nc, in_maps,
                               core_ids=list(range(cfg.num_devices)),
                               trace=trace)
    out = host_post(cfg, [res.results[c]["logits"]
                          for c in range(cfg.num_devices)])
    return out, res


def kernel(**inputs):
    x_mask = np.asarray(inputs["x_mask"], np.float32)
    y_mask = np.asarray(inputs["y_mask"], np.float32)
    bound = float(np.abs(np.asarray(inputs["w_att_v"], np.float32)).sum())
    shift = max(0.0, bound - 60.0)
    cfg = Cfg(with_mbias=not bool((x_mask == 1.0).all()),
              with_ymask=not bool((y_mask == 1.0).all()),
              exp_shift=shift)
    out, _ = run(cfg, inputs)
    return out


# revision 3
# speedup vs baseline: 1.0000x; 1.0000x over previous
"""Trainium2 Bass kernel for an attention seq2seq decoder (nn_Decoder).

Transposed-layout design, v2.

Reference math (per batch row b):
  att_h = eout @ wW.T                      (wb folded into state bias)
  scan over L-1 steps t:
    x = [emb[y_t], ctx]; h,c = LSTM(x, h, c; att_Wih, att_Whh, att_b)
    state = h @ vW.T + (vb + wb)
    scores = sum_d w_att_v[d] * tanh(state[d] + att_h[d,t']) + mbias
    alpha = softmax(scores); ctx = alpha @ eout
  att_fea = [h*ym, ctx*ym]
  dec scan: dh_t = LSTM(att_fea_t; dec_*)
  logit = ([att_fea, dh] * ym) @ cls_W.T + cls_b

Distribution: data-parallel over batch B=64 across 8 cores (8 rows/core),
all parameters replicated; the timestep scans stay local per core.

Device design notes (per core, 8 local rows in 2 groups of 4):
 - Everything recurrent lives TRANSPOSED: [128 (d%128), dchunk, b].  Gate
   matmuls, state matmul, score matmuls and context matmuls all use the
   batch (4) or a single column as the PE moving dimension, with the large
   tensors (weights, tanh tiles, eout) as the stationary operand.
 - gates come out of PE as gatesT [128 gate-dim, 8 chunks, b]; ACT tanh on
   [128, 32] replaces the old [4, 1024] stream.  sigmoid = 0.5(1+tanh(z/2))
   via host-halved i/f/o rows; hidden stored as 2h, cell as c/2.
 - att_fea is stored as [2h * ym, ctx * ym]; all consumers of the h part
   (dec_Wih, cls_W first/last thirds, vW) are pre-halved on the host, so
   the stored 2h needs no extra scaling and doubles as the recurrent h.
 - the big per-step tanh(state + att_h) over [d, T] is split across
   engines: ACT computes native tanh with the state add fused into the
   activation bias; DVE computes it via the addition formula
   tanh(s+a) = (ta+ts)/(1+ta*ts) with ta = tanh(att_h) precomputed in f32
   and the reciprocal evaluated by a custom 8-stage DVE op (NOT-seed +
   one Newton step); the numerator ta+ts is produced on the idle GPSIMD
   (Pool) engine.
 - scores come out of PE transposed [128 (t%128), b, tc] so exp is one
   [128, 32] ACT op and the exp columns feed the ctx matmuls directly
   (no alpha transposes).  Softmax sums: DVE reduce over tc + an
   all-ones f32 matmul over the t partitions.
"""

import numpy as np
import ml_dtypes
from dataclasses import dataclass

import concourse.bass as bass
import concourse.bacc as bacc
import concourse.tile as tile
import concourse.mybir as mybir
from concourse.masks import make_identity

F32 = mybir.dt.float32
BF16 = mybir.dt.bfloat16
AF = mybir.ActivationFunctionType
OP = mybir.AluOpType
AX = mybir.AxisListType
BF = ml_dtypes.bfloat16

D = 256  # model dim (layout hardcodes D == 2*128)


# ---------------------------------------------------------------------------
# custom DVE op: out = (Src0 + s0) * approx(1/Src1), Src1 = den in (0, 2).
# NOT-seed: z = den * bitcast(~den) lands in [-4.5, -4]; a relative-minimax
# linear fit 1/den ~= m*(B*z + A) on that interval gives ~1.7e-3 rel err.
# ---------------------------------------------------------------------------

TANH_RECIP_S1 = -0.0554592   # B
TANH_RECIP_IMM2 = -0.4714030  # A


def _register_tanh_recip():
    import concourse.dve_ops as dve_ops_mod
    from concourse.dve_ops import DveOp
    from concourse.dve_spec import AluOp, Bin, C0, C1, C2, Spec, Src0, \
        Src1, _has_src1, lower
    from concourse.dve_uop import DveOpSpec

    name = "TANH_RECIP_APPLY_ANT"
    if name in dve_ops_mod.CUSTOM_DVE_SPECS:
        return next(op for op in dve_ops_mod.OPS if op.name == name)

    m = Bin(AluOp.BITWISE_NOT, Src1, Src1)
    z = Bin(AluOp.MULTIPLY, Src1, m)
    f = Bin(AluOp.ADD, Bin(AluOp.MULTIPLY, z, C1), C2)
    r = Bin(AluOp.MULTIPLY, m, f)
    body = Bin(AluOp.MULTIPLY, Bin(AluOp.ADD, Src0, C0), r)

    def ref(in0, in1, c0, c1, c2):
        c0 = np.float32(c0) if isinstance(c0, float) else c0.astype(np.float32)
        c1 = np.float32(c1) if isinstance(c1, float) else c1.astype(np.float32)
        x = in1.astype(np.float32)
        m_ = (~x.view(np.int32)).view(np.float32)
        r_ = m_ * (x * m_ * c1 + np.float32(c2))
        return (in0.astype(np.float32) + c0) * r_

    spec = Spec(body=body, reference=ref)
    row = 0x1E
    assert row not in dve_ops_mod._SUB_OPCODE_FOR_NAME.values()
    dve_ops_mod._SUB_OPCODE_FOR_NAME[name] = row
    shas = {}
    for ver in ("v3", "v4"):
        s = DveOpSpec(name=name, opcode=row, uops=lower(spec, ver=ver),
                      rd1_en=_has_src1(spec))
        shas[ver] = s.sha(ver)
    op = DveOp(name, spec, subdim=False, uops_sha=shas)
    dve_ops_mod.OPS.append(op)
    dve_ops_mod.CUSTOM_DVE_SPECS[name] = spec
    return op


# unit assignment: units are (dc, bb) per group; 3 per group go to the
# DVE addition-formula path, the rest to native ACT tanh.
DVE_UNITS = {
    0: ((0, 2), (1, 2), (1, 3)),
    1: ((0, 2), (1, 2), (0, 3)),
}
# den producer per dve-unit index within the group: engine round-robin
DEN_ENGINE = ("pool", "pool", "dve")


@dataclass(frozen=True)
class Cfg:
    T: int = 1024          # encoder length
    L: int = 65            # decoder length (steps = L-1)
    V: int = 4235          # vocab
    BL: int = 8            # batch rows per core
    num_devices: int = 8
    with_mbias: bool = False
    with_ymask: bool = False
    exp_shift: float = 0.0   # constant subtracted inside exp (softmax-invariant)

    @property
    def NS(self):
        return self.L - 1

    @property
    def NT(self):
        return self.NS * self.BL  # total (t,g,b) rows

    @property
    def TC(self):
        return self.T // 128


def build_program(cfg: Cfg):
    NS, NT, T, V, TC = cfg.NS, cfg.NT, cfg.T, cfg.V, cfg.TC
    BL = cfg.BL
    assert BL == 8
    assert T % 128 == 0 and NT % 128 == 0
    NTC = NT // 128               # row chunks of pregates (4)
    MC = NT // 128                # classifier row chunks (4)
    NV = (V + 511) // 512         # vocab chunks (9)

    OP_T = _register_tanh_recip()

    # dve slot ids for ta_sb; act slot ids for atth_sb
    dve_slot = {}
    act_slot = {}
    for g in (0, 1):
        for u in DVE_UNITS[g]:
            dve_slot[(g,) + u] = len(dve_slot)
        for dc in range(2):
            for bb in range(4):
                if (g, dc, bb) not in dve_slot:
                    act_slot[(g, dc, bb)] = len(act_slot)
    NDVE = len(dve_slot)
    NACT = len(act_slot)

    nc = bacc.Bacc("TRN2", target_bir_lowering=False, debug=False,
                   num_devices=cfg.num_devices)

    def din(name, shape, dt=BF16):
        return nc.dram_tensor(name, shape, dt, kind="ExternalInput").ap()

    eout_d = din("eout_r", [128, BL, TC, D])
    embr_d = din("embr", [128, NTC, D])
    wihe_d = din("wihe", [128, 2, 1024])
    wihc_d = din("wihc", [128, 2, 1024])
    whh_d = din("whh", [128, 2, 1024])
    attb_d = din("attb", [1, 1024])
    ww_d = din("ww", [128, 2, 2, 128])
    vw_d = din("vw", [128, 2, 2, 128])
    biasvw_d = din("biasvw", [128, 2], F32)
    wv_d = din("wv", [128, 2])
    dwih_d = din("dwih", [128, 4, 1024])
    dwhh_d = din("dwhh", [128, 2, 1024])
    decb_d = din("decb", [1, 1024])
    cls_d = din("cls", [128, 6, V])
    clsb_d = din("clsb", [1, V])
    if cfg.with_ymask:
        ymT_d = din("ymT", [128, NS, BL], F32)
    if cfg.with_mbias:
        mbiasT_d = din("mbiasT", [128, BL, TC], F32)
    out_d = nc.dram_tensor("logits", [MC, 128, V], F32,
                           kind="ExternalOutput").ap()

    with tile.TileContext(nc) as tc:
        import contextlib
        stack = contextlib.ExitStack()
        with stack:
            singles = stack.enter_context(tc.tile_pool(name="singles", bufs=1))

            # ---------- persistent SBUF ----------
            eout_sb = singles.tile([128, BL, TC, D], BF16)
            atth_sb = singles.tile([128, NACT, T], BF16)
            if NDVE:
                ta_sb = singles.tile([128, NDVE, T], F32)
            pregates_sb = singles.tile([128, NTC, 1024], BF16)
            affT_sb = singles.tile([128, 4, NT], BF16)
            dhT_sb = singles.tile([128, 2, NT], BF16)
            wihc_sb = singles.tile([128, 2, 1024], BF16)
            whh_sb = singles.tile([128, 2, 1024], BF16)
            attb_sb = singles.tile([1, 1024], BF16)
            vw_sb = singles.tile([128, 2, 2, 128], BF16)
            biasvw_sb = singles.tile([128, 2], F32)
            wv_sb = singles.tile([128, 2], BF16)
            dwih_sb = singles.tile([128, 4, 1024], BF16)
            dwhh_sb = singles.tile([128, 2, 1024], BF16)
            decb_sb = singles.tile([1, 1024], BF16)
            clsb_sb = singles.tile([1, V], BF16)
            ident = singles.tile([128, 128], BF16)
            ones_sb = singles.tile([1, 128], BF16)
            onesf_sb = singles.tile([128, 128], F32)

            state_sb = singles.tile([128, 2, BL], F32)
            ts_sb = singles.tile([128, 2, BL], F32)
            cT_sb = singles.tile([128, 2, BL], F32)      # att c/2, cols g4+bb
            cdT_sb = singles.tile([128, 2, BL], F32)     # dec c/2
            if cfg.with_ymask:
                hT_sb = singles.tile([128, 2, BL], BF16)     # recurrent 2h
                ctxT_sb = singles.tile([128, 2, BL], BF16)   # recurrent ctx
                hdT_sb = singles.tile([128, 2, BL], BF16)    # dec recurrent 2h
                ymT_sb = singles.tile([128, NS, BL], F32)
            if cfg.with_mbias:
                mbiasT_sb = singles.tile([128, BL, TC], F32)

            # ---------- input DMAs ----------
            for dst, src in [
                (wihc_sb, wihc_d), (whh_sb, whh_d), (attb_sb, attb_d),
                (vw_sb, vw_d), (biasvw_sb, biasvw_d), (wv_sb, wv_d),
                (dwih_sb, dwih_d), (dwhh_sb, dwhh_d),
                (decb_sb, decb_d), (clsb_sb, clsb_d),
            ]:
                nc.sync.dma_start(out=dst[:], in_=src)
            if cfg.with_ymask:
                nc.sync.dma_start(out=ymT_sb[:], in_=ymT_d)
            if cfg.with_mbias:
                nc.sync.dma_start(out=mbiasT_sb[:], in_=mbiasT_d)
            for b_ in range(BL):
                nc.sync.dma_start(out=eout_sb[:, b_, :, :],
                                  in_=eout_d[:, b_, :, :])

            make_identity(nc, ident[:])
            nc.vector.memset(ones_sb[:], 1.0)
            nc.vector.memset(onesf_sb[:], 1.0)
            nc.vector.memset(cT_sb[:], 0.0)
            nc.vector.memset(cdT_sb[:], 0.0)
            nc.vector.memset(state_sb[:], 0.0)
            nc.vector.memset(ts_sb[:], 0.0)
            if cfg.with_ymask:
                nc.vector.memset(hT_sb[:], 0.0)
                nc.vector.memset(ctxT_sb[:], 0.0)
                nc.vector.memset(hdT_sb[:], 0.0)

            # ---------- prep phase ----------
            with tc.tile_pool(name="prep_ps", bufs=3, space="PSUM") as pps, \
                 tc.tile_pool(name="prep_ps2", bufs=3, space="PSUM") as pps2, \
                 tc.tile_pool(name="prep_sb", bufs=2) as psb:
                embr_sb = psb.tile([128, NTC, D], BF16, bufs=1)
                embT_sb = psb.tile([128, 2, NT], BF16, bufs=1)
                wihe_sb = psb.tile([128, 2, 1024], BF16, bufs=1)
                ww_sb = psb.tile([128, 2, 2, 128], BF16, bufs=1)
                nc.sync.dma_start(out=embr_sb[:], in_=embr_d)
                nc.sync.dma_start(out=wihe_sb[:], in_=wihe_d)
                nc.sync.dma_start(out=ww_sb[:], in_=ww_d)
                # embT: transpose embr row-chunks -> [d, row]
                for m in range(NTC):
                    for ch in range(2):
                        tp = pps.tile([128, 128], BF16, tag="tp")
                        nc.tensor.transpose(
                            tp[:], embr_sb[:, m, ch * 128:(ch + 1) * 128],
                            ident[:])
                        nc.vector.tensor_copy(
                            embT_sb[:, ch, m * 128:(m + 1) * 128], tp[:])
                # att pregates = embed @ WihE.T + att_b  -> [row, 1024]
                for m in range(NTC):
                    for half in range(2):
                        hs = slice(half * 512, (half + 1) * 512)
                        gp = pps2.tile([128, 512], F32, tag="gp")
                        nc.tensor.matmul(gp[:], ones_sb[0:1, :],
                                         attb_sb[0:1, hs],
                                         start=True, stop=False)
                        for kc in range(2):
                            nc.tensor.matmul(
                                gp[:],
                                embT_sb[:, kc, m * 128:(m + 1) * 128],
                                wihe_sb[:, kc, hs],
                                start=False, stop=(kc == 1))
                        nc.vector.tensor_copy(pregates_sb[:, m, hs], gp[:])
                # att_h transposed [d, t] per b: eoutT then ww @ eoutT
                for b in range(BL):
                    g, bb = divmod(b, 4)
                    eoutT_b = psb.tile([128, 2, T], BF16, tag="eoutT")
                    for t_c in range(TC):
                        for ch in range(2):
                            tp2 = pps.tile([128, 128], BF16, tag="tp")
                            nc.tensor.transpose(
                                tp2[:],
                                eout_sb[:, b, t_c, ch * 128:(ch + 1) * 128],
                                ident[:])
                            dst = eoutT_b[:, ch, t_c * 128:(t_c + 1) * 128]
                            if (t_c * 2 + ch) % 2 == 0:
                                nc.vector.tensor_copy(dst, tp2[:])
                            else:
                                nc.scalar.copy(dst, tp2[:])
                    for dc in range(2):
                        is_dve = (g, dc, bb) in dve_slot
                        for n in range(T // 512):
                            ap = pps2.tile([128, 512], F32, tag="gp")
                            for kc in range(2):
                                nc.tensor.matmul(
                                    ap[:],
                                    ww_sb[:, kc, dc, :],
                                    eoutT_b[:, kc, n * 512:(n + 1) * 512],
                                    start=(kc == 0), stop=(kc == 1))
                            ns = slice(n * 512, (n + 1) * 512)
                            bvw = biasvw_sb[:, dc:dc + 1]
                            if is_dve:
                                # ta = tanh(att_h + vb + wb), f32
                                sl = dve_slot[(g, dc, bb)]
                                nc.scalar.activation(ta_sb[:, sl, ns], ap[:],
                                                     AF.Tanh, bias=bvw)
                            else:
                                # atth' = att_h + vb + wb, bf16
                                asl = act_slot[(g, dc, bb)]
                                nc.vector.tensor_scalar(
                                    atth_sb[:, asl, ns], ap[:], bvw, None,
                                    OP.add)

            # ---------- scan phase ----------
            with tc.tile_pool(name="psG", bufs=2, space="PSUM") as psG, \
                 tc.tile_pool(name="psS", bufs=2, space="PSUM") as psS, \
                 tc.tile_pool(name="psM", bufs=2, space="PSUM") as psM, \
                 tc.tile_pool(name="psL", bufs=2, space="PSUM") as psL, \
                 tc.tile_pool(name="scsb", bufs=4) as scsb, \
                 tc.tile_pool(name="thp", bufs=4) as thp, \
                 tc.tile_pool(name="nump", bufs=3) as nump, \
                 tc.tile_pool(name="cwp", bufs=3) as cwp, \
                 tc.tile_pool(name="csb", bufs=2) as csb:

                def rc_prev(g, t):
                    return (t - 1) * 8 + g * 4

                def att_gates(g, t):
                    """gatesT [128, 8, 4] for group g step t (PE)."""
                    r0 = t * 8 + g * 4
                    m, p0 = divmod(r0, 128)
                    rp = rc_prev(g, t)
                    gp = psG.tile([128, 8, 4], F32, tag="g")
                    for gc in range(8):
                        gs = slice(gc * 128, (gc + 1) * 128)
                        first = (t == 0 and not cfg.with_ymask)
                        nc.tensor.matmul(gp[:, gc, :],
                                         pregates_sb[:, m, gs],
                                         ident[:, p0:p0 + 4],
                                         start=True, stop=first)
                        if first:
                            continue
                        if cfg.with_ymask:
                            ctx_rhs = [ctxT_sb[:, kc, g * 4:g * 4 + 4]
                                       for kc in range(2)]
                            h_rhs = [hT_sb[:, kc, g * 4:g * 4 + 4]
                                     for kc in range(2)]
                        else:
                            ctx_rhs = [affT_sb[:, 2 + kc, rp:rp + 4]
                                       for kc in range(2)]
                            h_rhs = [affT_sb[:, kc, rp:rp + 4]
                                     for kc in range(2)]
                        for kc in range(2):
                            nc.tensor.matmul(gp[:, gc, :],
                                             wihc_sb[:, kc, gs], ctx_rhs[kc],
                                             start=False, stop=False)
                        for kc in range(2):
                            nc.tensor.matmul(gp[:, gc, :],
                                             whh_sb[:, kc, gs], h_rhs[kc],
                                             start=False, stop=(kc == 1))
                    return gp

                def lstm_tail1(g, t, gp):
                    """tanh gates + c/h update for group g step t."""
                    g4 = g * 4
                    r0 = t * 8 + g * 4
                    tg = scsb.tile([128, 8, 4], BF16, tag="tg")
                    nc.scalar.activation(tg[:], gp[:], AF.Tanh)
                    ti = tg[:, 0:2, :]
                    tf = tg[:, 2:4, :]
                    tgg = tg[:, 4:6, :]
                    to = tg[:, 6:8, :]
                    cc = cT_sb[:, :, g4:g4 + 4]
                    aT = scsb.tile([128, 2, 4], F32, tag="aT")
                    bT = scsb.tile([128, 2, 4], F32, tag="bT")
                    tT = scsb.tile([128, 2, 4], F32, tag="tT")
                    nc.vector.scalar_tensor_tensor(aT[:], tf, 1.0, cc,
                                                   OP.add, OP.mult)
                    nc.vector.scalar_tensor_tensor(bT[:], ti, 1.0, tgg,
                                                   OP.add, OP.mult)
                    nc.vector.scalar_tensor_tensor(tT[:], bT[:], 0.5, aT[:],
                                                   OP.mult, OP.add)
                    tcb = scsb.tile([128, 2, 4], BF16, tag="tcb")
                    nc.scalar.activation(tcb[:], tT[:], AF.Tanh)
                    # hH = 2h = (to+1)*tanh(c)
                    if cfg.with_ymask:
                        hh = hT_sb[:, :, g4:g4 + 4]
                        nc.vector.scalar_tensor_tensor(hh, to, 1.0, tcb[:],
                                                       OP.add, OP.mult)
                        ym = ymT_sb[:, t, g4:g4 + 4]
                        ymb = bass.AP(tensor=ym.tensor, offset=ym.offset,
                                      ap=[ym.ap[0], [0, 2], [1, 4]])
                        nc.vector.tensor_tensor(affT_sb[:, 0:2, r0:r0 + 4],
                                                hh, ymb, OP.mult)
                    else:
                        nc.vector.scalar_tensor_tensor(
                            affT_sb[:, 0:2, r0:r0 + 4], to, 1.0, tcb[:],
                            OP.add, OP.mult)
                    nc.vector.tensor_scalar_mul(cc, tT[:], 0.5)

                def lstm_tail2(g, t):
                    """state = vw @ hH (bias lives in atth/ta); ts = tanh."""
                    g4 = g * 4
                    r0 = t * 8 + g * 4
                    stp = psM.tile([128, 24], F32, tag="m")
                    for mc2 in range(2):
                        for kc in range(2):
                            h_rhs = (hT_sb[:, kc, g4:g4 + 4] if cfg.with_ymask
                                     else affT_sb[:, kc, r0:r0 + 4])
                            nc.tensor.matmul(
                                stp[:, mc2 * 4:mc2 * 4 + 4],
                                vw_sb[:, kc, mc2, :], h_rhs,
                                start=(kc == 0), stop=(kc == 1))
                    stv = bass.AP(tensor=stp.tensor, offset=stp.offset,
                                  ap=[stp.ap[0], [4, 2], [1, 4]])
                    nc.scalar.activation(ts_sb[:, :, g4:g4 + 4], stv, AF.Tanh)
                    return stv

                def unit_act(g, dc, bb, t):
                    c = g * 4 + bb
                    asl = act_slot[(g, dc, bb)]
                    th = thp.tile([128, T], BF16, tag="th", bufs=12)
                    nc.scalar.activation(th[:], atth_sb[:, asl, :],
                                         AF.Tanh,
                                         bias=state_sb[:, dc, c:c + 1])
                    return th

                def scores_b(sc, bb, th0, th1):
                    for t_c in range(TC):
                        nc.tensor.matmul(sc[:, bb, t_c:t_c + 1],
                                         th0[:, t_c * 128:(t_c + 1) * 128],
                                         wv_sb[:, 0:1], start=True, stop=False)
                        nc.tensor.matmul(sc[:, bb, t_c:t_c + 1],
                                         th1[:, t_c * 128:(t_c + 1) * 128],
                                         wv_sb[:, 1:2], start=False, stop=True)

                def emit_dens(g, t, stv):
                    """dens on Pool/DVE right after ts; state psum copy."""
                    g4 = g * 4
                    dve = DVE_UNITS[g]
                    dens = {}
                    for k, (dc, bb) in enumerate(dve):
                        c = g4 + bb
                        sl = dve_slot[(g, dc, bb)]
                        den = nump.tile([128, T], F32, tag="nm", bufs=6)
                        eng = (nc.gpsimd if DEN_ENGINE[k] == "pool"
                               else nc.vector)
                        eng.tensor_scalar(den[:], ta_sb[:, sl, :],
                                          ts_sb[:, dc, c:c + 1], 1.0,
                                          OP.mult, OP.add)
                        dens[(dc, bb)] = den
                    # state for the ACT units' fused bias (plain psum copy)
                    nc.vector.tensor_copy(state_sb[:, :, g4:g4 + 4], stv)
                    return dens

                def big_tanh(g, t, dens):
                    """units + score matmuls; returns the sc psum tile."""
                    g4 = g * 4
                    sc = psS.tile([128, 4, TC], F32, tag="sc")
                    dve = DVE_UNITS[g]
                    ths = {}
                    for (dc, bb) in dve:
                        c = g4 + bb
                        sl = dve_slot[(g, dc, bb)]
                        th = thp.tile([128, T], BF16, tag="th", bufs=12)
                        nc.vector._custom_dve(
                            OP_T, out=th[:], in0=ta_sb[:, sl, :],
                            in1=dens[(dc, bb)][:],
                            s0=ts_sb[:, dc, c:c + 1],
                            s1=TANH_RECIP_S1, imm2=TANH_RECIP_IMM2)
                        ths[(dc, bb)] = th
                    order = sorted(
                        [(bb, dc) for dc in range(2) for bb in range(4)],
                        key=lambda p: ((p[1], p[0]) in dve, p[0]))
                    done = set()
                    for bb, dc in order:
                        if (dc, bb) not in ths:
                            ths[(dc, bb)] = unit_act(g, dc, bb, t)
                        done.add((dc, bb))
                        if (0, bb) in done and (1, bb) in done:
                            scores_b(sc, bb, ths[(0, bb)], ths[(1, bb)])
                    return sc

                def finish_a(g, t, sc):
                    """exp + per-partition partial softmax sums."""
                    g4 = g * 4
                    exp_sb = scsb.tile([128, 4, TC], BF16, tag="exp")
                    if cfg.with_mbias:
                        scm = scsb.tile([128, 4, TC], F32, tag="scm")
                        nc.vector.tensor_tensor(scm[:], sc[:],
                                                mbiasT_sb[:, g4:g4 + 4, :],
                                                OP.add)
                        nc.scalar.activation(exp_sb[:], scm[:], AF.Exp,
                                             bias=float(-cfg.exp_shift))
                    else:
                        nc.scalar.activation(exp_sb[:], sc[:], AF.Exp,
                                             bias=float(-cfg.exp_shift))
                    sp = scsb.tile([128, 4], F32, tag="sp")
                    nc.vector.tensor_reduce(sp[:], exp_sb[:], AX.X, OP.add)
                    return exp_sb, sp

                def finish_b(g, t, exp_sb, sp):
                    """ctx matmuls, softmax total, ctx store."""
                    g4 = g * 4
                    r0 = t * 8 + g * 4
                    combo = psM.tile([128, 24], F32, tag="m")
                    for dch in range(2):
                        for bb in range(4):
                            col = combo[:, dch * 4 + bb:dch * 4 + bb + 1]
                            for t_c in range(TC):
                                nc.tensor.matmul(
                                    col,
                                    eout_sb[:, g4 + bb, t_c,
                                            dch * 128:(dch + 1) * 128],
                                    exp_sb[:, bb, t_c:t_c + 1],
                                    start=(t_c == 0), stop=(t_c == TC - 1))
                    es = combo[:, 16:20]
                    nc.tensor.matmul(es, onesf_sb[:], sp[:],
                                     start=True, stop=True)
                    rcp = scsb.tile([128, 4], F32, tag="rcp")
                    nc.vector.reciprocal(rcp[:], es)
                    ctxv = bass.AP(tensor=combo.tensor, offset=combo.offset,
                                   ap=[combo.ap[0], [4, 2], [1, 4]])
                    rcb = bass.AP(tensor=rcp.tensor, offset=rcp.offset,
                                  ap=[rcp.ap[0], [0, 2], [1, 4]])
                    if cfg.with_ymask:
                        ct = ctxT_sb[:, :, g4:g4 + 4]
                        nc.vector.tensor_tensor(ct, ctxv, rcb, OP.mult)
                        ym = ymT_sb[:, t, g4:g4 + 4]
                        ymb = bass.AP(tensor=ym.tensor, offset=ym.offset,
                                      ap=[ym.ap[0], [0, 2], [1, 4]])
                        nc.vector.tensor_tensor(affT_sb[:, 2:4, r0:r0 + 4],
                                                ct, ymb, OP.mult)
                    else:
                        nc.vector.tensor_tensor(affT_sb[:, 2:4, r0:r0 + 4],
                                                ctxv, rcb, OP.mult)

                def dec_step(u):
                    """dec LSTM step u over all 8 rows."""
                    r0 = u * 8
                    rp = r0 - 8
                    gp = psG.tile([128, 8, 8], F32, tag="g")
                    skip_h = (u == 0 and not cfg.with_ymask)
                    for gc in range(8):
                        gs = slice(gc * 128, (gc + 1) * 128)
                        nc.tensor.matmul(gp[:, gc, :],
                                         decb_sb[0:1, gs], ones_sb[0:1, 0:8],
                                         start=True, stop=False)
                        for ch in range(4):
                            nc.tensor.matmul(gp[:, gc, :],
                                             dwih_sb[:, ch, gs],
                                             affT_sb[:, ch, r0:r0 + 8],
                                             start=False,
                                             stop=(skip_h and ch == 3))
                        if skip_h:
                            continue
                        for kc in range(2):
                            h_rhs = (hdT_sb[:, kc, :] if cfg.with_ymask
                                     else dhT_sb[:, kc, rp:rp + 8])
                            nc.tensor.matmul(gp[:, gc, :],
                                             dwhh_sb[:, kc, gs], h_rhs,
                                             start=False, stop=(kc == 1))
                    tg = scsb.tile([128, 8, 8], BF16, tag="dtg")
                    nc.scalar.activation(tg[:], gp[:], AF.Tanh)
                    ti = tg[:, 0:2, :]
                    tf = tg[:, 2:4, :]
                    tgg = tg[:, 4:6, :]
                    to = tg[:, 6:8, :]
                    aT = scsb.tile([128, 2, 8], F32, tag="daT")
                    bT = scsb.tile([128, 2, 8], F32, tag="dbT")
                    tT = scsb.tile([128, 2, 8], F32, tag="dtT")
                    nc.vector.scalar_tensor_tensor(aT[:], tf, 1.0, cdT_sb[:],
                                                   OP.add, OP.mult)
                    nc.vector.scalar_tensor_tensor(bT[:], ti, 1.0, tgg,
                                                   OP.add, OP.mult)
                    nc.vector.scalar_tensor_tensor(tT[:], bT[:], 0.5, aT[:],
                                                   OP.mult, OP.add)
                    tcb = scsb.tile([128, 2, 8], BF16, tag="dtcb")
                    nc.scalar.activation(tcb[:], tT[:], AF.Tanh)
                    if cfg.with_ymask:
                        nc.vector.scalar_tensor_tensor(hdT_sb[:], to, 1.0,
                                                       tcb[:], OP.add, OP.mult)
                        ym = ymT_sb[:, u, :]
                        ymb = bass.AP(tensor=ym.tensor, offset=ym.offset,
                                      ap=[ym.ap[0], [0, 2], [1, 8]])
                        nc.vector.tensor_tensor(dhT_sb[:, :, r0:r0 + 8],
                                                hdT_sb[:], ymb, OP.mult)
                    else:
                        nc.vector.scalar_tensor_tensor(
                            dhT_sb[:, :, r0:r0 + 8], to, 1.0, tcb[:],
                            OP.add, OP.mult)
                    nc.vector.tensor_scalar_mul(cdT_sb[:], tT[:], 0.5)

                def cls_dma(k):
                    """prefetch classifier weights for unit k (one DMA)."""
                    nv = k % NV
                    nn = min(512, V - nv * 512)
                    ns = slice(nv * 512, nv * 512 + nn)
                    wt = cwp.tile([128, 6, 512], BF16, tag="wt")
                    nc.sync.dma_start(out=wt[:, :, 0:nn], in_=cls_d[:, :, ns])
                    return wt

                cls_out = []   # (k, lp) awaiting copy-out

                def cls_flush():
                    while cls_out:
                        k, lp = cls_out.pop(0)
                        m, nv = divmod(k, NV)
                        nn = min(512, V - nv * 512)
                        ns = slice(nv * 512, nv * 512 + nn)
                        lsb = csb.tile([128, 512], F32, tag="lsb")
                        nc.vector.tensor_copy(lsb[:, 0:nn], lp[:, 0:nn])
                        nc.sync.dma_start(out=out_d[m, :, ns],
                                          in_=lsb[:, 0:nn])

                def cls_mm(k, wt):
                    """classifier matmuls for unit k = m*NV + nv."""
                    m, nv = divmod(k, NV)
                    ms = slice(m * 128, (m + 1) * 128)
                    nn = min(512, V - nv * 512)
                    ns = slice(nv * 512, nv * 512 + nn)
                    cls_flush()   # previous unit's psum is long done
                    lp = psL.tile([128, 512], F32, tag="lp")
                    nc.tensor.matmul(lp[:, 0:nn], ones_sb[0:1, :],
                                     clsb_sb[0:1, ns],
                                     start=True, stop=False)
                    for ch in range(4):
                        nc.tensor.matmul(lp[:, 0:nn], affT_sb[:, ch, ms],
                                         wt[:, ch, 0:nn],
                                         start=False, stop=False)
                    for ch in range(2):
                        nc.tensor.matmul(lp[:, 0:nn], dhT_sb[:, ch, ms],
                                         wt[:, 4 + ch, 0:nn],
                                         start=False, stop=(ch == 1))
                    cls_out.append((k, lp))

                # ---------------- main loop ----------------
                pend = {}
                dec_done = 0
                cls_done = 0
                cls_pre = 0
                cls_q = []

                def cls_ready(k):
                    m = k // NV
                    # rows m*128..(m+1)*128 need dec steps up to 16(m+1)
                    return m < MC - 1 and dec_done >= 16 * (m + 1)

                pend_gates = {}
                for t in range(NS):
                    for g in (0, 1):
                        og = 1 - g
                        # PE order per half: [state] [ctx og + esum] [hoisted
                        # og gates for its next step] [unit scores] [dec] [cls]
                        gp = pend_gates.pop(g, None)
                        if gp is None:
                            gp = att_gates(g, t)
                        fo = None
                        if og in pend:
                            pt, psc = pend.pop(og)
                            fo = (pt, *finish_a(og, pt, psc))
                        lstm_tail1(g, t, gp)
                        stv = lstm_tail2(g, t)
                        if fo is not None:
                            finish_b(og, *fo)
                            nt = fo[0] + 1
                            if nt < NS:
                                pend_gates[og] = att_gates(og, nt)
                        pend[g] = (t, big_tanh(g, t, emit_dens(g, t, stv)))
                        if dec_done < t:
                            dec_step(dec_done)
                            dec_done += 1
                        # classifier: prefetch weights ahead, matmul when
                        # the rows' dec outputs are complete
                        if (cls_pre < MC * NV and cls_pre - cls_done < 2
                                and cls_ready(max(cls_pre - 1, 0))):
                            cls_q.append(cls_dma(cls_pre))
                            cls_pre += 1
                        if cls_q and cls_ready(cls_done):
                            cls_mm(cls_done, cls_q.pop(0))
                            cls_done += 1
                for g in (0, 1):
                    if g in pend:
                        pt, psc = pend.pop(g)
                        finish_b(g, pt, *finish_a(g, pt, psc))
                while dec_done < NS:
                    dec_step(dec_done)
                    dec_done += 1
                while cls_done < MC * NV:
                    while cls_pre < MC * NV and cls_pre - cls_done < 2:
                        cls_q.append(cls_dma(cls_pre))
                        cls_pre += 1
                    cls_mm(cls_done, cls_q.pop(0))
                    cls_done += 1
                cls_flush()

    nc.compile()
    return nc


# ---------------------------------------------------------------------------
# host marshaling
# ---------------------------------------------------------------------------

def host_prep_shared(cfg: Cfg, emb, att_Wih, att_Whh, att_b, wW, wb, vW, vb,
                     w_att_v, dec_Wih, dec_Whh, dec_b, cls_W, cls_b):
    """Weight preprocessing shared by all cores."""
    f = np.float32
    att_Wih = np.asarray(att_Wih, f).copy()
    att_Whh = np.asarray(att_Whh, f).copy()
    att_b = np.asarray(att_b, f).copy()
    dec_Wih = np.asarray(dec_Wih, f).copy()
    dec_Whh = np.asarray(dec_Whh, f).copy()
    dec_b = np.asarray(dec_b, f).copy()
    cls_W = np.asarray(cls_W, f).copy()
    # sigmoid(z) = 0.5*(1+tanh(z/2)): halve i,f,o rows (gate order i,f,g,o)
    ifo = np.r_[0:512, 768:1024]
    for W in (att_Wih, dec_Wih, att_Whh, dec_Whh):
        W[ifo] *= 0.5
    for bvec in (att_b, dec_b):
        bvec[ifo] *= 0.5
    # hidden state stored as 2h: halve all 2h-consuming weights
    att_Whh *= 0.5
    dec_Whh *= 0.5
    vW05 = np.asarray(vW, f) * 0.5
    dec_Wih[:, 0:256] *= 0.5       # att_fea h part stored as 2h*ym
    cls_W[:, 0:256] *= 0.5         # idem
    cls_W[:, 512:768] *= 0.5       # dec h stored as 2h*ym

    def pack_kn(WT, kc):  # [K, N] -> [128, kc, N]
        K, N = WT.shape
        assert K == kc * 128
        return np.ascontiguousarray(
            WT.reshape(kc, 128, N).transpose(1, 0, 2)).astype(BF)

    wihe = pack_kn(att_Wih[:, 0:256].T, 2)
    wihc = pack_kn(att_Wih[:, 256:512].T, 2)
    whh = pack_kn(att_Whh.T, 2)
    dwih = pack_kn(dec_Wih.T, 4)
    dwhh = pack_kn(dec_Whh.T, 2)

    def pack_kmn(WT):  # [256, 256] -> [128, kc2, mc2, 128]
        return np.ascontiguousarray(
            WT.reshape(2, 128, 2, 128).transpose(1, 0, 2, 3)).astype(BF)

    vw = pack_kmn(vW05.T)
    ww = pack_kmn(np.asarray(wW, f).T)
    biasvw = np.ascontiguousarray(
        (np.asarray(vb, f) + np.asarray(wb, f)).reshape(2, 128).T)
    wv = np.ascontiguousarray(
        np.asarray(w_att_v, f).reshape(2, 128).T).astype(BF)   # [128, dc]
    cls = np.ascontiguousarray(
        cls_W.T.reshape(6, 128, cfg.V).transpose(1, 0, 2)).astype(BF)
    shared = dict(
        wihe=wihe, wihc=wihc, whh=whh,
        attb=att_b.reshape(1, 1024).astype(BF),
        vw=vw, ww=ww, biasvw=biasvw.astype(f), wv=wv,
        dwih=dwih, dwhh=dwhh, decb=dec_b.reshape(1, 1024).astype(BF),
        cls=cls, clsb=np.asarray(cls_b, f).reshape(1, cfg.V).astype(BF),
    )
    return shared


def host_prep_core(cfg: Cfg, c, eout, x_mask, y, y_mask, emb, shared):
    """Per-core input shards. b rows c*BL .. c*BL+BL."""
    f = np.float32
    BL, T, NS, TC, NT = cfg.BL, cfg.T, cfg.NS, cfg.TC, cfg.NT
    NTC = NT // 128
    sl = slice(c * BL, (c + 1) * BL)
    e = np.asarray(eout[sl], f)                       # [BL, T, D]
    eout_r = np.ascontiguousarray(
        e.reshape(BL, TC, 128, D).transpose(2, 0, 1, 3)).astype(BF)
    yv = np.asarray(y[sl])                            # [BL, L]
    embed = np.asarray(emb, f)[yv[:, :-1]]            # [BL, NS, D]
    embed_r = np.ascontiguousarray(
        embed.transpose(1, 0, 2).reshape(NT, D))      # [(t,b), D]
    embr = np.ascontiguousarray(
        embed_r.reshape(NTC, 128, D).transpose(1, 0, 2)).astype(BF)
    d = dict(shared)
    d.update(eout_r=eout_r, embr=embr)
    if cfg.with_ymask:
        ym = np.asarray(y_mask[sl], f)[:, 1:]         # [BL, NS]
        ymT = np.broadcast_to(ym.T[None], (128, NS, BL))
        d["ymT"] = np.ascontiguousarray(ymT).astype(f)
    if cfg.with_mbias:
        mb = (np.asarray(x_mask[sl], f)[..., 0] - 1.0) * 1e30  # [BL, T]
        mbT = mb.T.reshape(TC, 128, BL).transpose(1, 2, 0)     # [128, BL, TC]
        d["mbiasT"] = np.ascontiguousarray(mbT).astype(f)
    return d


def host_post(cfg: Cfg, outs):
    """Reassemble [MC,128,V] per-core row-major (t,b) results -> [B, NS, V]."""
    parts = []
    for o in outs:
        lg = o.reshape(cfg.NT, cfg.V).reshape(cfg.NS, cfg.BL, cfg.V)
        parts.append(np.ascontiguousarray(lg.transpose(1, 0, 2)))
    return np.concatenate(parts, axis=0)


_PROG_CACHE = {}


def _get_program(cfg: Cfg):
    if cfg not in _PROG_CACHE:
        _PROG_CACHE[cfg] = build_program(cfg)
    return _PROG_CACHE[cfg]


def run(cfg: Cfg, inputs, trace=False):
    from concourse.bass_utils import run_bass_kernel_spmd
    nc = _get_program(cfg)
    shared = host_prep_shared(
        cfg, inputs["emb"], inputs["att_Wih"], inputs["att_Whh"],
        inputs["att_b"], inputs["wW"], inputs["wb"], inputs["vW"],
        inputs["vb"], inputs["w_att_v"], inputs["dec_Wih"],
        inputs["dec_Whh"], inputs["dec_b"], inputs["cls_W"], inputs["cls_b"])
    in_maps = [
        host_prep_core(cfg, c, inputs["eout"], inputs["x_mask"], inputs["y"],
                       inputs["y_mask"], inputs["emb"], shared)
        for c in range(cfg.num_devices)
    ]
    res = run_bass_kernel_spmd(nc, in_maps,
                               core_ids=list(range(cfg.num_devices)),
                               trace=trace)
    out = host_post(cfg, [res.results[c]["logits"]
                          for c in range(cfg.num_devices)])
    return out, res


def kernel(**inputs):
    x_mask = np.asarray(inputs["x_mask"], np.float32)
    y_mask = np.asarray(inputs["y_mask"], np.float32)
    bound = float(np.abs(np.asarray(inputs["w_att_v"], np.float32)).sum())
    shift = max(0.0, bound - 60.0)
    cfg = Cfg(with_mbias=not bool((x_mask == 1.0).all()),
              with_ymask=not bool((y_mask == 1.0).all()),
              exp_shift=shift)
    out, _ = run(cfg, inputs)
    return out


# revision 4
# speedup vs baseline: 1.0069x; 1.0069x over previous
"""Trainium2 Bass kernel for an attention seq2seq decoder (nn_Decoder).

Transposed-layout design, v2.

Reference math (per batch row b):
  att_h = eout @ wW.T                      (wb folded into state bias)
  scan over L-1 steps t:
    x = [emb[y_t], ctx]; h,c = LSTM(x, h, c; att_Wih, att_Whh, att_b)
    state = h @ vW.T + (vb + wb)
    scores = sum_d w_att_v[d] * tanh(state[d] + att_h[d,t']) + mbias
    alpha = softmax(scores); ctx = alpha @ eout
  att_fea = [h*ym, ctx*ym]
  dec scan: dh_t = LSTM(att_fea_t; dec_*)
  logit = ([att_fea, dh] * ym) @ cls_W.T + cls_b

Distribution: data-parallel over batch B=64 across 8 cores (8 rows/core),
all parameters replicated; the timestep scans stay local per core.

Device design notes (per core, 8 local rows in 2 groups of 4):
 - Everything recurrent lives TRANSPOSED: [128 (d%128), dchunk, b].  Gate
   matmuls, state matmul, score matmuls and context matmuls all use the
   batch (4) or a single column as the PE moving dimension, with the large
   tensors (weights, tanh tiles, eout) as the stationary operand.
 - gates come out of PE as gatesT [128 gate-dim, 8 chunks, b]; ACT tanh on
   [128, 32] replaces the old [4, 1024] stream.  sigmoid = 0.5(1+tanh(z/2))
   via host-halved i/f/o rows; hidden stored as 2h, cell as c/2.
 - att_fea is stored as [2h * ym, ctx * ym]; all consumers of the h part
   (dec_Wih, cls_W first/last thirds, vW) are pre-halved on the host, so
   the stored 2h needs no extra scaling and doubles as the recurrent h.
 - the big per-step tanh(state + att_h) over [d, T] is split across
   engines: ACT computes native tanh with the state add fused into the
   activation bias; DVE computes it via the addition formula
   tanh(s+a) = (ta+ts)/(1+ta*ts) with ta = tanh(att_h) precomputed in f32
   and the reciprocal evaluated by a custom 8-stage DVE op (NOT-seed +
   one Newton step); the numerator ta+ts is produced on the idle GPSIMD
   (Pool) engine.
 - scores come out of PE transposed [128 (t%128), b, tc] so exp is one
   [128, 32] ACT op and the exp columns feed the ctx matmuls directly
   (no alpha transposes).  Softmax sums: DVE reduce over tc + an
   all-ones f32 matmul over the t partitions.
"""

import numpy as np
import ml_dtypes
from dataclasses import dataclass

import concourse.bass as bass
import concourse.bacc as bacc
import concourse.tile as tile
import concourse.mybir as mybir
from concourse.masks import make_identity

F32 = mybir.dt.float32
BF16 = mybir.dt.bfloat16
AF = mybir.ActivationFunctionType
OP = mybir.AluOpType
AX = mybir.AxisListType
BF = ml_dtypes.bfloat16

D = 256  # model dim (layout hardcodes D == 2*128)


# ---------------------------------------------------------------------------
# custom DVE op: out = (Src0 + s0) * approx(1/Src1), Src1 = den in (0, 2).
# NOT-seed: z = den * bitcast(~den) lands in [-4.5, -4]; a relative-minimax
# linear fit 1/den ~= m*(B*z + A) on that interval gives ~1.7e-3 rel err.
# ---------------------------------------------------------------------------

TANH_RECIP_S1 = -0.0554592   # B
TANH_RECIP_IMM2 = -0.4714030  # A


def _register_tanh_recip():
    import concourse.dve_ops as dve_ops_mod
    from concourse.dve_ops import DveOp
    from concourse.dve_spec import AluOp, Bin, C0, C1, C2, Spec, Src0, \
        Src1, _has_src1, lower
    from concourse.dve_uop import DveOpSpec

    name = "TANH_RECIP_APPLY_ANT"
    if name in dve_ops_mod.CUSTOM_DVE_SPECS:
        return next(op for op in dve_ops_mod.OPS if op.name == name)

    m = Bin(AluOp.BITWISE_NOT, Src1, Src1)
    z = Bin(AluOp.MULTIPLY, Src1, m)
    f = Bin(AluOp.ADD, Bin(AluOp.MULTIPLY, z, C1), C2)
    r = Bin(AluOp.MULTIPLY, m, f)
    body = Bin(AluOp.MULTIPLY, Bin(AluOp.ADD, Src0, C0), r)

    def ref(in0, in1, c0, c1, c2):
        c0 = np.float32(c0) if isinstance(c0, float) else c0.astype(np.float32)
        c1 = np.float32(c1) if isinstance(c1, float) else c1.astype(np.float32)
        x = in1.astype(np.float32)
        m_ = (~x.view(np.int32)).view(np.float32)
        r_ = m_ * (x * m_ * c1 + np.float32(c2))
        return (in0.astype(np.float32) + c0) * r_

    spec = Spec(body=body, reference=ref)
    row = 0x1E
    assert row not in dve_ops_mod._SUB_OPCODE_FOR_NAME.values()
    dve_ops_mod._SUB_OPCODE_FOR_NAME[name] = row
    shas = {}
    for ver in ("v3", "v4"):
        s = DveOpSpec(name=name, opcode=row, uops=lower(spec, ver=ver),
                      rd1_en=_has_src1(spec))
        shas[ver] = s.sha(ver)
    op = DveOp(name, spec, subdim=False, uops_sha=shas)
    dve_ops_mod.OPS.append(op)
    dve_ops_mod.CUSTOM_DVE_SPECS[name] = spec
    return op


# unit assignment: units are (dc, bb) per group; 3 per group go to the
# DVE addition-formula path, the rest to native ACT tanh.
DVE_UNITS = {
    0: ((0, 2), (1, 2), (1, 3)),
    1: ((0, 2), (1, 2), (0, 3)),
}
# den producer per dve-unit index within the group: engine round-robin
DEN_ENGINE = ("pool", "pool", "dve")


@dataclass(frozen=True)
class Cfg:
    T: int = 1024          # encoder length
    L: int = 65            # decoder length (steps = L-1)
    V: int = 4235          # vocab
    BL: int = 8            # batch rows per core
    num_devices: int = 8
    with_mbias: bool = False
    with_ymask: bool = False
    exp_shift: float = 0.0   # constant subtracted inside exp (softmax-invariant)

    @property
    def NS(self):
        return self.L - 1

    @property
    def NT(self):
        return self.NS * self.BL  # total (t,g,b) rows

    @property
    def TC(self):
        return self.T // 128


def build_program(cfg: Cfg):
    NS, NT, T, V, TC = cfg.NS, cfg.NT, cfg.T, cfg.V, cfg.TC
    BL = cfg.BL
    assert BL == 8
    assert T % 128 == 0 and NT % 128 == 0
    NTC = NT // 128               # row chunks of pregates (4)
    MC = NT // 128                # classifier row chunks (4)
    NV = (V + 511) // 512         # vocab chunks (9)

    OP_T = _register_tanh_recip()

    # dve slot ids for ta_sb; act slot ids for atth_sb
    dve_slot = {}
    act_slot = {}
    for g in (0, 1):
        for u in DVE_UNITS[g]:
            dve_slot[(g,) + u] = len(dve_slot)
        for dc in range(2):
            for bb in range(4):
                if (g, dc, bb) not in dve_slot:
                    act_slot[(g, dc, bb)] = len(act_slot)
    NDVE = len(dve_slot)
    NACT = len(act_slot)

    nc = bacc.Bacc("TRN2", target_bir_lowering=False, debug=False,
                   num_devices=cfg.num_devices)

    def din(name, shape, dt=BF16):
        return nc.dram_tensor(name, shape, dt, kind="ExternalInput").ap()

    eout_d = din("eout_r", [128, BL, TC, D])
    embr_d = din("embr", [128, NTC, D])
    wihe_d = din("wihe", [128, 2, 1024])
    wihc_d = din("wihc", [128, 2, 1024])
    whh_d = din("whh", [128, 2, 1024])
    attb_d = din("attb", [1, 1024])
    ww_d = din("ww", [128, 2, 2, 128])
    vw_d = din("vw", [128, 2, 2, 128])
    biasvw_d = din("biasvw", [128, 2], F32)
    wv_d = din("wv", [128, 2])
    dwih_d = din("dwih", [128, 4, 1024])
    dwhh_d = din("dwhh", [128, 2, 1024])
    decb_d = din("decb", [1, 1024])
    cls_d = din("cls", [128, 6, V])
    clsb_d = din("clsb", [1, V])
    if cfg.with_ymask:
        ymT_d = din("ymT", [128, NS, BL], F32)
    if cfg.with_mbias:
        mbiasT_d = din("mbiasT", [128, BL, TC], F32)
    out_d = nc.dram_tensor("logits", [MC, 128, V], F32,
                           kind="ExternalOutput").ap()

    with tile.TileContext(nc) as tc:
        import contextlib
        stack = contextlib.ExitStack()
        with stack:
            singles = stack.enter_context(tc.tile_pool(name="singles", bufs=1))

            # ---------- persistent SBUF ----------
            eout_sb = singles.tile([128, BL, TC, D], BF16)
            atth_sb = singles.tile([128, NACT, T], BF16)
            if NDVE:
                ta_sb = singles.tile([128, NDVE, T], F32)
            pregates_sb = singles.tile([128, NTC, 1024], BF16)
            affT_sb = singles.tile([128, 4, NT], BF16)
            dhT_sb = singles.tile([128, 2, NT], BF16)
            wihc_sb = singles.tile([128, 2, 1024], BF16)
            whh_sb = singles.tile([128, 2, 1024], BF16)
            attb_sb = singles.tile([1, 1024], BF16)
            vw_sb = singles.tile([128, 2, 2, 128], BF16)
            biasvw_sb = singles.tile([128, 2], F32)
            wv_sb = singles.tile([128, 2], BF16)
            dwih_sb = singles.tile([128, 4, 1024], BF16)
            dwhh_sb = singles.tile([128, 2, 1024], BF16)
            decb_sb = singles.tile([1, 1024], BF16)
            clsb_sb = singles.tile([1, V], BF16)
            ident = singles.tile([128, 128], BF16)
            ones_sb = singles.tile([1, 128], BF16)
            onesf_sb = singles.tile([128, 128], F32)

            state_sb = singles.tile([128, 2, BL], F32)
            ts_sb = singles.tile([128, 2, BL], F32)
            cT_sb = singles.tile([128, 2, BL], F32)      # att c/2, cols g4+bb
            cdT_sb = singles.tile([128, 2, BL], F32)     # dec c/2
            if cfg.with_ymask:
                hT_sb = singles.tile([128, 2, BL], BF16)     # recurrent 2h
                ctxT_sb = singles.tile([128, 2, BL], BF16)   # recurrent ctx
                hdT_sb = singles.tile([128, 2, BL], BF16)    # dec recurrent 2h
                ymT_sb = singles.tile([128, NS, BL], F32)
            if cfg.with_mbias:
                mbiasT_sb = singles.tile([128, BL, TC], F32)

            # ---------- input DMAs ----------
            for dst, src in [
                (wihc_sb, wihc_d), (whh_sb, whh_d), (attb_sb, attb_d),
                (vw_sb, vw_d), (biasvw_sb, biasvw_d), (wv_sb, wv_d),
                (dwih_sb, dwih_d), (dwhh_sb, dwhh_d),
                (decb_sb, decb_d), (clsb_sb, clsb_d),
            ]:
                nc.sync.dma_start(out=dst[:], in_=src)
            if cfg.with_ymask:
                nc.sync.dma_start(out=ymT_sb[:], in_=ymT_d)
            if cfg.with_mbias:
                nc.sync.dma_start(out=mbiasT_sb[:], in_=mbiasT_d)
            for b_ in range(BL):
                nc.sync.dma_start(out=eout_sb[:, b_, :, :],
                                  in_=eout_d[:, b_, :, :])

            make_identity(nc, ident[:])
            nc.vector.memset(ones_sb[:], 1.0)
            nc.vector.memset(onesf_sb[:], 1.0)
            nc.vector.memset(cT_sb[:], 0.0)
            nc.vector.memset(cdT_sb[:], 0.0)
            nc.vector.memset(state_sb[:], 0.0)
            nc.vector.memset(ts_sb[:], 0.0)
            if cfg.with_ymask:
                nc.vector.memset(hT_sb[:], 0.0)
                nc.vector.memset(ctxT_sb[:], 0.0)
                nc.vector.memset(hdT_sb[:], 0.0)

            # ---------- prep phase ----------
            with tc.tile_pool(name="prep_ps", bufs=3, space="PSUM") as pps, \
                 tc.tile_pool(name="prep_ps2", bufs=3, space="PSUM") as pps2, \
                 tc.tile_pool(name="prep_sb", bufs=2) as psb:
                embr_sb = psb.tile([128, NTC, D], BF16, bufs=1)
                embT_sb = psb.tile([128, 2, NT], BF16, bufs=1)
                wihe_sb = psb.tile([128, 2, 1024], BF16, bufs=1)
                ww_sb = psb.tile([128, 2, 2, 128], BF16, bufs=1)
                nc.sync.dma_start(out=embr_sb[:], in_=embr_d)
                nc.sync.dma_start(out=wihe_sb[:], in_=wihe_d)
                nc.sync.dma_start(out=ww_sb[:], in_=ww_d)
                # embT: transpose embr row-chunks -> [d, row]
                for m in range(NTC):
                    for ch in range(2):
                        tp = pps.tile([128, 128], BF16, tag="tp")
                        nc.tensor.transpose(
                            tp[:], embr_sb[:, m, ch * 128:(ch + 1) * 128],
                            ident[:])
                        nc.vector.tensor_copy(
                            embT_sb[:, ch, m * 128:(m + 1) * 128], tp[:])
                # att pregates = embed @ WihE.T + att_b  -> [row, 1024]
                for m in range(NTC):
                    for half in range(2):
                        hs = slice(half * 512, (half + 1) * 512)
                        gp = pps2.tile([128, 512], F32, tag="gp")
                        nc.tensor.matmul(gp[:], ones_sb[0:1, :],
                                         attb_sb[0:1, hs],
                                         start=True, stop=False)
                        for kc in range(2):
                            nc.tensor.matmul(
                                gp[:],
                                embT_sb[:, kc, m * 128:(m + 1) * 128],
                                wihe_sb[:, kc, hs],
                                start=False, stop=(kc == 1))
                        nc.vector.tensor_copy(pregates_sb[:, m, hs], gp[:])
                # att_h transposed [d, t] per b: eoutT then ww @ eoutT
                for b in range(BL):
                    g, bb = divmod(b, 4)
                    eoutT_b = psb.tile([128, 2, T], BF16, tag="eoutT")
                    for t_c in range(TC):
                        for ch in range(2):
                            tp2 = pps.tile([128, 128], BF16, tag="tp")
                            nc.tensor.transpose(
                                tp2[:],
                                eout_sb[:, b, t_c, ch * 128:(ch + 1) * 128],
                                ident[:])
                            dst = eoutT_b[:, ch, t_c * 128:(t_c + 1) * 128]
                            if (t_c * 2 + ch) % 2 == 0:
                                nc.vector.tensor_copy(dst, tp2[:])
                            else:
                                nc.scalar.copy(dst, tp2[:])
                    for dc in range(2):
                        is_dve = (g, dc, bb) in dve_slot
                        for n in range(T // 512):
                            ap = pps2.tile([128, 512], F32, tag="gp")
                            for kc in range(2):
                                nc.tensor.matmul(
                                    ap[:],
                                    ww_sb[:, kc, dc, :],
                                    eoutT_b[:, kc, n * 512:(n + 1) * 512],
                                    start=(kc == 0), stop=(kc == 1))
                            ns = slice(n * 512, (n + 1) * 512)
                            bvw = biasvw_sb[:, dc:dc + 1]
                            if is_dve:
                                # ta = tanh(att_h + vb + wb), f32
                                sl = dve_slot[(g, dc, bb)]
                                nc.scalar.activation(ta_sb[:, sl, ns], ap[:],
                                                     AF.Tanh, bias=bvw)
                            else:
                                # atth' = att_h + vb + wb, bf16
                                asl = act_slot[(g, dc, bb)]
                                nc.vector.tensor_scalar(
                                    atth_sb[:, asl, ns], ap[:], bvw, None,
                                    OP.add)

            # ---------- scan phase ----------
            with tc.tile_pool(name="psG", bufs=2, space="PSUM") as psG, \
                 tc.tile_pool(name="psS", bufs=2, space="PSUM") as psS, \
                 tc.tile_pool(name="psM", bufs=2, space="PSUM") as psM, \
                 tc.tile_pool(name="psL", bufs=2, space="PSUM") as psL, \
                 tc.tile_pool(name="scsb", bufs=4) as scsb, \
                 tc.tile_pool(name="thp", bufs=4) as thp, \
                 tc.tile_pool(name="nump", bufs=3) as nump, \
                 tc.tile_pool(name="cwp", bufs=3) as cwp, \
                 tc.tile_pool(name="csb", bufs=2) as csb:

                def rc_prev(g, t):
                    return (t - 1) * 8 + g * 4

                def att_gates(g, t):
                    """gatesT [128, 8, 4] for group g step t (PE)."""
                    r0 = t * 8 + g * 4
                    m, p0 = divmod(r0, 128)
                    rp = rc_prev(g, t)
                    gp = psG.tile([128, 8, 4], F32, tag="g")
                    for gc in range(8):
                        gs = slice(gc * 128, (gc + 1) * 128)
                        first = (t == 0 and not cfg.with_ymask)
                        nc.tensor.matmul(gp[:, gc, :],
                                         pregates_sb[:, m, gs],
                                         ident[:, p0:p0 + 4],
                                         start=True, stop=first)
                        if first:
                            continue
                        if cfg.with_ymask:
                            ctx_rhs = [ctxT_sb[:, kc, g * 4:g * 4 + 4]
                                       for kc in range(2)]
                            h_rhs = [hT_sb[:, kc, g * 4:g * 4 + 4]
                                     for kc in range(2)]
                        else:
                            ctx_rhs = [affT_sb[:, 2 + kc, rp:rp + 4]
                                       for kc in range(2)]
                            h_rhs = [affT_sb[:, kc, rp:rp + 4]
                                     for kc in range(2)]
                        for kc in range(2):
                            nc.tensor.matmul(gp[:, gc, :],
                                             wihc_sb[:, kc, gs], ctx_rhs[kc],
                                             start=False, stop=False)
                        for kc in range(2):
                            nc.tensor.matmul(gp[:, gc, :],
                                             whh_sb[:, kc, gs], h_rhs[kc],
                                             start=False, stop=(kc == 1))
                    return gp

                def lstm_tail1(g, t, gp):
                    """tanh gates + c/h update for group g step t."""
                    g4 = g * 4
                    r0 = t * 8 + g * 4
                    tg = scsb.tile([128, 8, 4], BF16, tag="tg")
                    nc.scalar.activation(tg[:], gp[:], AF.Tanh)
                    ti = tg[:, 0:2, :]
                    tf = tg[:, 2:4, :]
                    tgg = tg[:, 4:6, :]
                    to = tg[:, 6:8, :]
                    cc = cT_sb[:, :, g4:g4 + 4]
                    aT = scsb.tile([128, 2, 4], F32, tag="aT")
                    bT = scsb.tile([128, 2, 4], F32, tag="bT")
                    tT = scsb.tile([128, 2, 4], F32, tag="tT")
                    nc.vector.scalar_tensor_tensor(aT[:], tf, 1.0, cc,
                                                   OP.add, OP.mult)
                    nc.vector.scalar_tensor_tensor(bT[:], ti, 1.0, tgg,
                                                   OP.add, OP.mult)
                    nc.vector.scalar_tensor_tensor(tT[:], bT[:], 0.5, aT[:],
                                                   OP.mult, OP.add)
                    tcb = scsb.tile([128, 2, 4], BF16, tag="tcb")
                    nc.scalar.activation(tcb[:], tT[:], AF.Tanh)
                    # hH = 2h = (to+1)*tanh(c)
                    if cfg.with_ymask:
                        hh = hT_sb[:, :, g4:g4 + 4]
                        nc.vector.scalar_tensor_tensor(hh, to, 1.0, tcb[:],
                                                       OP.add, OP.mult)
                        ym = ymT_sb[:, t, g4:g4 + 4]
                        ymb = bass.AP(tensor=ym.tensor, offset=ym.offset,
                                      ap=[ym.ap[0], [0, 2], [1, 4]])
                        nc.vector.tensor_tensor(affT_sb[:, 0:2, r0:r0 + 4],
                                                hh, ymb, OP.mult)
                    else:
                        nc.vector.scalar_tensor_tensor(
                            affT_sb[:, 0:2, r0:r0 + 4], to, 1.0, tcb[:],
                            OP.add, OP.mult)
                    nc.vector.tensor_scalar_mul(cc, tT[:], 0.5)

                def lstm_tail2(g, t):
                    """state = vw @ hH (bias lives in atth/ta); ts = tanh."""
                    g4 = g * 4
                    r0 = t * 8 + g * 4
                    stp = psM.tile([128, 24], F32, tag="m")
                    for mc2 in range(2):
                        for kc in range(2):
                            h_rhs = (hT_sb[:, kc, g4:g4 + 4] if cfg.with_ymask
                                     else affT_sb[:, kc, r0:r0 + 4])
                            nc.tensor.matmul(
                                stp[:, mc2 * 4:mc2 * 4 + 4],
                                vw_sb[:, kc, mc2, :], h_rhs,
                                start=(kc == 0), stop=(kc == 1))
                    stv = bass.AP(tensor=stp.tensor, offset=stp.offset,
                                  ap=[stp.ap[0], [4, 2], [1, 4]])
                    nc.scalar.activation(ts_sb[:, :, g4:g4 + 4], stv, AF.Tanh)
                    return stv

                def unit_act(g, dc, bb, t):
                    c = g * 4 + bb
                    asl = act_slot[(g, dc, bb)]
                    th = thp.tile([128, T], BF16, tag="th", bufs=12)
                    nc.scalar.activation(th[:], atth_sb[:, asl, :],
                                         AF.Tanh,
                                         bias=state_sb[:, dc, c:c + 1])
                    return th

                def scores_b(sc, bb, th0, th1):
                    for t_c in range(TC):
                        nc.tensor.matmul(sc[:, bb, t_c:t_c + 1],
                                         th0[:, t_c * 128:(t_c + 1) * 128],
                                         wv_sb[:, 0:1], start=True, stop=False)
                        nc.tensor.matmul(sc[:, bb, t_c:t_c + 1],
                                         th1[:, t_c * 128:(t_c + 1) * 128],
                                         wv_sb[:, 1:2], start=False, stop=True)

                def emit_dens(g, t, stv):
                    """dens on Pool/DVE right after ts; state psum copy."""
                    g4 = g * 4
                    dve = DVE_UNITS[g]
                    dens = {}
                    for k, (dc, bb) in enumerate(dve):
                        c = g4 + bb
                        sl = dve_slot[(g, dc, bb)]
                        den = nump.tile([128, T], F32, tag="nm", bufs=6)
                        eng = (nc.gpsimd if DEN_ENGINE[k] == "pool"
                               else nc.vector)
                        eng.tensor_scalar(den[:], ta_sb[:, sl, :],
                                          ts_sb[:, dc, c:c + 1], 1.0,
                                          OP.mult, OP.add)
                        dens[(dc, bb)] = den
                    # state for the ACT units' fused bias: copy on ACT so
                    # the unit tanhs never wait on the DVE queue for it
                    nc.scalar.copy(state_sb[:, :, g4:g4 + 4], stv)
                    return dens

                def big_tanh(g, t, dens):
                    """units + score matmuls; returns the sc psum tile."""
                    g4 = g * 4
                    sc = psS.tile([128, 4, TC], F32, tag="sc")
                    dve = DVE_UNITS[g]
                    ths = {}
                    for (dc, bb) in dve:
                        c = g4 + bb
                        sl = dve_slot[(g, dc, bb)]
                        th = thp.tile([128, T], BF16, tag="th", bufs=12)
                        nc.vector._custom_dve(
                            OP_T, out=th[:], in0=ta_sb[:, sl, :],
                            in1=dens[(dc, bb)][:],
                            s0=ts_sb[:, dc, c:c + 1],
                            s1=TANH_RECIP_S1, imm2=TANH_RECIP_IMM2)
                        ths[(dc, bb)] = th
                    order = sorted(
                        [(bb, dc) for dc in range(2) for bb in range(4)],
                        key=lambda p: ((p[1], p[0]) in dve, p[0]))
                    done = set()
                    for bb, dc in order:
                        if (dc, bb) not in ths:
                            ths[(dc, bb)] = unit_act(g, dc, bb, t)
                        done.add((dc, bb))
                        if (0, bb) in done and (1, bb) in done:
                            scores_b(sc, bb, ths[(0, bb)], ths[(1, bb)])
                    return sc

                def finish_a(g, t, sc):
                    """exp + per-partition partial softmax sums."""
                    g4 = g * 4
                    exp_sb = scsb.tile([128, 4, TC], BF16, tag="exp")
                    if cfg.with_mbias:
                        scm = scsb.tile([128, 4, TC], F32, tag="scm")
                        nc.vector.tensor_tensor(scm[:], sc[:],
                                                mbiasT_sb[:, g4:g4 + 4, :],
                                                OP.add)
                        nc.scalar.activation(exp_sb[:], scm[:], AF.Exp,
                                             bias=float(-cfg.exp_shift))
                    else:
                        nc.scalar.activation(exp_sb[:], sc[:], AF.Exp,
                                             bias=float(-cfg.exp_shift))
                    sp = scsb.tile([128, 4], F32, tag="sp")
                    nc.vector.tensor_reduce(sp[:], exp_sb[:], AX.X, OP.add)
                    return exp_sb, sp

                def finish_b(g, t, exp_sb, sp):
                    """ctx matmuls, softmax total, ctx store."""
                    g4 = g * 4
                    r0 = t * 8 + g * 4
                    combo = psM.tile([128, 24], F32, tag="m")
                    for dch in range(2):
                        for bb in range(4):
                            col = combo[:, dch * 4 + bb:dch * 4 + bb + 1]
                            for t_c in range(TC):
                                nc.tensor.matmul(
                                    col,
                                    eout_sb[:, g4 + bb, t_c,
                                            dch * 128:(dch + 1) * 128],
                                    exp_sb[:, bb, t_c:t_c + 1],
                                    start=(t_c == 0), stop=(t_c == TC - 1))
                    es = combo[:, 16:20]
                    nc.tensor.matmul(es, onesf_sb[:], sp[:],
                                     start=True, stop=True)
                    rcp = scsb.tile([128, 4], F32, tag="rcp")
                    nc.vector.reciprocal(rcp[:], es)
                    ctxv = bass.AP(tensor=combo.tensor, offset=combo.offset,
                                   ap=[combo.ap[0], [4, 2], [1, 4]])
                    rcb = bass.AP(tensor=rcp.tensor, offset=rcp.offset,
                                  ap=[rcp.ap[0], [0, 2], [1, 4]])
                    if cfg.with_ymask:
                        ct = ctxT_sb[:, :, g4:g4 + 4]
                        nc.vector.tensor_tensor(ct, ctxv, rcb, OP.mult)
                        ym = ymT_sb[:, t, g4:g4 + 4]
                        ymb = bass.AP(tensor=ym.tensor, offset=ym.offset,
                                      ap=[ym.ap[0], [0, 2], [1, 4]])
                        nc.vector.tensor_tensor(affT_sb[:, 2:4, r0:r0 + 4],
                                                ct, ymb, OP.mult)
                    else:
                        nc.vector.tensor_tensor(affT_sb[:, 2:4, r0:r0 + 4],
                                                ctxv, rcb, OP.mult)

                def dec_step(u):
                    """dec LSTM step u over all 8 rows."""
                    r0 = u * 8
                    rp = r0 - 8
                    gp = psG.tile([128, 8, 8], F32, tag="g")
                    skip_h = (u == 0 and not cfg.with_ymask)
                    for gc in range(8):
                        gs = slice(gc * 128, (gc + 1) * 128)
                        nc.tensor.matmul(gp[:, gc, :],
                                         decb_sb[0:1, gs], ones_sb[0:1, 0:8],
                                         start=True, stop=False)
                        for ch in range(4):
                            nc.tensor.matmul(gp[:, gc, :],
                                             dwih_sb[:, ch, gs],
                                             affT_sb[:, ch, r0:r0 + 8],
                                             start=False,
                                             stop=(skip_h and ch == 3))
                        if skip_h:
                            continue
                        for kc in range(2):
                            h_rhs = (hdT_sb[:, kc, :] if cfg.with_ymask
                                     else dhT_sb[:, kc, rp:rp + 8])
                            nc.tensor.matmul(gp[:, gc, :],
                                             dwhh_sb[:, kc, gs], h_rhs,
                                             start=False, stop=(kc == 1))
                    tg = scsb.tile([128, 8, 8], BF16, tag="dtg")
                    nc.scalar.activation(tg[:], gp[:], AF.Tanh)
                    ti = tg[:, 0:2, :]
                    tf = tg[:, 2:4, :]
                    tgg = tg[:, 4:6, :]
                    to = tg[:, 6:8, :]
                    aT = scsb.tile([128, 2, 8], F32, tag="daT")
                    bT = scsb.tile([128, 2, 8], F32, tag="dbT")
                    tT = scsb.tile([128, 2, 8], F32, tag="dtT")
                    nc.vector.scalar_tensor_tensor(aT[:], tf, 1.0, cdT_sb[:],
                                                   OP.add, OP.mult)
                    nc.vector.scalar_tensor_tensor(bT[:], ti, 1.0, tgg,
                                                   OP.add, OP.mult)
                    nc.vector.scalar_tensor_tensor(tT[:], bT[:], 0.5, aT[:],
                                                   OP.mult, OP.add)
                    tcb = scsb.tile([128, 2, 8], BF16, tag="dtcb")
                    nc.scalar.activation(tcb[:], tT[:], AF.Tanh)
                    if cfg.with_ymask:
                        nc.vector.scalar_tensor_tensor(hdT_sb[:], to, 1.0,
                                                       tcb[:], OP.add, OP.mult)
                        ym = ymT_sb[:, u, :]
                        ymb = bass.AP(tensor=ym.tensor, offset=ym.offset,
                                      ap=[ym.ap[0], [0, 2], [1, 8]])
                        nc.vector.tensor_tensor(dhT_sb[:, :, r0:r0 + 8],
                                                hdT_sb[:], ymb, OP.mult)
                    else:
                        nc.vector.scalar_tensor_tensor(
                            dhT_sb[:, :, r0:r0 + 8], to, 1.0, tcb[:],
                            OP.add, OP.mult)
                    nc.vector.tensor_scalar_mul(cdT_sb[:], tT[:], 0.5)

                def cls_dma(k):
                    """prefetch classifier weights for unit k (one DMA)."""
                    nv = k % NV
                    nn = min(512, V - nv * 512)
                    ns = slice(nv * 512, nv * 512 + nn)
                    wt = cwp.tile([128, 6, 512], BF16, tag="wt")
                    nc.sync.dma_start(out=wt[:, :, 0:nn], in_=cls_d[:, :, ns])
                    return wt

                cls_out = []   # (k, lp) awaiting copy-out

                def cls_flush():
                    while cls_out:
                        k, lp = cls_out.pop(0)
                        m, nv = divmod(k, NV)
                        nn = min(512, V - nv * 512)
                        ns = slice(nv * 512, nv * 512 + nn)
                        lsb = csb.tile([128, 512], F32, tag="lsb")
                        nc.vector.tensor_copy(lsb[:, 0:nn], lp[:, 0:nn])
                        nc.sync.dma_start(out=out_d[m, :, ns],
                                          in_=lsb[:, 0:nn])

                def cls_mm(k, wt):
                    """classifier matmuls for unit k = m*NV + nv."""
                    m, nv = divmod(k, NV)
                    ms = slice(m * 128, (m + 1) * 128)
                    nn = min(512, V - nv * 512)
                    ns = slice(nv * 512, nv * 512 + nn)
                    cls_flush()   # previous unit's psum is long done
                    lp = psL.tile([128, 512], F32, tag="lp")
                    nc.tensor.matmul(lp[:, 0:nn], ones_sb[0:1, :],
                                     clsb_sb[0:1, ns],
                                     start=True, stop=False)
                    for ch in range(4):
                        nc.tensor.matmul(lp[:, 0:nn], affT_sb[:, ch, ms],
                                         wt[:, ch, 0:nn],
                                         start=False, stop=False)
                    for ch in range(2):
                        nc.tensor.matmul(lp[:, 0:nn], dhT_sb[:, ch, ms],
                                         wt[:, 4 + ch, 0:nn],
                                         start=False, stop=(ch == 1))
                    cls_out.append((k, lp))

                # ---------------- main loop ----------------
                pend = {}
                dec_done = 0
                cls_done = 0
                cls_pre = 0
                cls_q = []

                def cls_ready(k):
                    m = k // NV
                    # rows m*128..(m+1)*128 need dec steps up to 16(m+1)
                    return m < MC - 1 and dec_done >= 16 * (m + 1)

                pend_gates = {}
                for t in range(NS):
                    for g in (0, 1):
                        og = 1 - g
                        # PE order per half: [state] [ctx og + esum] [hoisted
                        # og gates for its next step] [unit scores] [dec] [cls]
                        gp = pend_gates.pop(g, None)
                        if gp is None:
                            gp = att_gates(g, t)
                        lstm_tail1(g, t, gp)
                        fo = None
                        if og in pend:
                            pt, psc = pend.pop(og)
                            fo = (pt, *finish_a(og, pt, psc))
                        stv = lstm_tail2(g, t)
                        if fo is not None:
                            finish_b(og, *fo)
                            nt = fo[0] + 1
                            if nt < NS:
                                pend_gates[og] = att_gates(og, nt)
                        pend[g] = (t, big_tanh(g, t, emit_dens(g, t, stv)))
                        if dec_done < t:
                            dec_step(dec_done)
                            dec_done += 1
                        # classifier: prefetch weights ahead, matmul when
                        # the rows' dec outputs are complete
                        if cls_pre < MC * NV and cls_pre - cls_done < 3:
                            cls_q.append(cls_dma(cls_pre))
                            cls_pre += 1
                        if cls_q and cls_ready(cls_done):
                            cls_mm(cls_done, cls_q.pop(0))
                            cls_done += 1
                for g in (0, 1):
                    if g in pend:
                        pt, psc = pend.pop(g)
                        finish_b(g, pt, *finish_a(g, pt, psc))
                while dec_done < NS:
                    dec_step(dec_done)
                    dec_done += 1
                while cls_done < MC * NV:
                    while cls_pre < MC * NV and cls_pre - cls_done < 3:
                        cls_q.append(cls_dma(cls_pre))
                        cls_pre += 1
                    cls_mm(cls_done, cls_q.pop(0))
                    cls_done += 1
                cls_flush()

    nc.compile()
    return nc


# ---------------------------------------------------------------------------
# host marshaling
# ---------------------------------------------------------------------------

def host_prep_shared(cfg: Cfg, emb, att_Wih, att_Whh, att_b, wW, wb, vW, vb,
                     w_att_v, dec_Wih, dec_Whh, dec_b, cls_W, cls_b):
    """Weight preprocessing shared by all cores."""
    f = np.float32
    att_Wih = np.asarray(att_Wih, f).copy()
    att_Whh = np.asarray(att_Whh, f).copy()
    att_b = np.asarray(att_b, f).copy()
    dec_Wih = np.asarray(dec_Wih, f).copy()
    dec_Whh = np.asarray(dec_Whh, f).copy()
    dec_b = np.asarray(dec_b, f).copy()
    cls_W = np.asarray(cls_W, f).copy()
    # sigmoid(z) = 0.5*(1+tanh(z/2)): halve i,f,o rows (gate order i,f,g,o)
    ifo = np.r_[0:512, 768:1024]
    for W in (att_Wih, dec_Wih, att_Whh, dec_Whh):
        W[ifo] *= 0.5
    for bvec in (att_b, dec_b):
        bvec[ifo] *= 0.5
    # hidden state stored as 2h: halve all 2h-consuming weights
    att_Whh *= 0.5
    dec_Whh *= 0.5
    vW05 = np.asarray(vW, f) * 0.5
    dec_Wih[:, 0:256] *= 0.5       # att_fea h part stored as 2h*ym
    cls_W[:, 0:256] *= 0.5         # idem
    cls_W[:, 512:768] *= 0.5       # dec h stored as 2h*ym

    def pack_kn(WT, kc):  # [K, N] -> [128, kc, N]
        K, N = WT.shape
        assert K == kc * 128
        return np.ascontiguousarray(
            WT.reshape(kc, 128, N).transpose(1, 0, 2)).astype(BF)

    wihe = pack_kn(att_Wih[:, 0:256].T, 2)
    wihc = pack_kn(att_Wih[:, 256:512].T, 2)
    whh = pack_kn(att_Whh.T, 2)
    dwih = pack_kn(dec_Wih.T, 4)
    dwhh = pack_kn(dec_Whh.T, 2)

    def pack_kmn(WT):  # [256, 256] -> [128, kc2, mc2, 128]
        return np.ascontiguousarray(
            WT.reshape(2, 128, 2, 128).transpose(1, 0, 2, 3)).astype(BF)

    vw = pack_kmn(vW05.T)
    ww = pack_kmn(np.asarray(wW, f).T)
    biasvw = np.ascontiguousarray(
        (np.asarray(vb, f) + np.asarray(wb, f)).reshape(2, 128).T)
    wv = np.ascontiguousarray(
        np.asarray(w_att_v, f).reshape(2, 128).T).astype(BF)   # [128, dc]
    cls = np.ascontiguousarray(
        cls_W.T.reshape(6, 128, cfg.V).transpose(1, 0, 2)).astype(BF)
    shared = dict(
        wihe=wihe, wihc=wihc, whh=whh,
        attb=att_b.reshape(1, 1024).astype(BF),
        vw=vw, ww=ww, biasvw=biasvw.astype(f), wv=wv,
        dwih=dwih, dwhh=dwhh, decb=dec_b.reshape(1, 1024).astype(BF),
        cls=cls, clsb=np.asarray(cls_b, f).reshape(1, cfg.V).astype(BF),
    )
    return shared


def host_prep_core(cfg: Cfg, c, eout, x_mask, y, y_mask, emb, shared):
    """Per-core input shards. b rows c*BL .. c*BL+BL."""
    f = np.float32
    BL, T, NS, TC, NT = cfg.BL, cfg.T, cfg.NS, cfg.TC, cfg.NT
    NTC = NT // 128
    sl = slice(c * BL, (c + 1) * BL)
    e = np.asarray(eout[sl], f)                       # [BL, T, D]
    eout_r = np.ascontiguousarray(
        e.reshape(BL, TC, 128, D).transpose(2, 0, 1, 3)).astype(BF)
    yv = np.asarray(y[sl])                            # [BL, L]
    embed = np.asarray(emb, f)[yv[:, :-1]]            # [BL, NS, D]
    embed_r = np.ascontiguousarray(
        embed.transpose(1, 0, 2).reshape(NT, D))      # [(t,b), D]
    embr = np.ascontiguousarray(
        embed_r.reshape(NTC, 128, D).transpose(1, 0, 2)).astype(BF)
    d = dict(shared)
    d.update(eout_r=eout_r, embr=embr)
    if cfg.with_ymask:
        ym = np.asarray(y_mask[sl], f)[:, 1:]         # [BL, NS]
        ymT = np.broadcast_to(ym.T[None], (128, NS, BL))
        d["ymT"] = np.ascontiguousarray(ymT).astype(f)
    if cfg.with_mbias:
        mb = (np.asarray(x_mask[sl], f)[..., 0] - 1.0) * 1e30  # [BL, T]
        mbT = mb.T.reshape(TC, 128, BL).transpose(1, 2, 0)     # [128, BL, TC]
        d["mbiasT"] = np.ascontiguousarray(mbT).astype(f)
    return d


def host_post(cfg: Cfg, outs):
    """Reassemble [MC,128,V] per-core row-major (t,b) results -> [B, NS, V]."""
    parts = []
    for o in outs:
        lg = o.reshape(cfg.NT, cfg.V).reshape(cfg.NS, cfg.BL, cfg.V)
        parts.append(np.ascontiguousarray(lg.transpose(1, 0, 2)))
    return np.concatenate(parts, axis=0)


_PROG_CACHE = {}


def _get_program(cfg: Cfg):
    if cfg not in _PROG_CACHE:
        _PROG_CACHE[cfg] = build_program(cfg)
    return _PROG_CACHE[cfg]


def run(cfg: Cfg, inputs, trace=False):
    from concourse.bass_utils import run_bass_kernel_spmd
    nc = _get_program(cfg)
    shared = host_prep_shared(
        cfg, inputs["emb"], inputs["att_Wih"], inputs["att_Whh"],
        inputs["att_b"], inputs["wW"], inputs["wb"], inputs["vW"],
        inputs["vb"], inputs["w_att_v"], inputs["dec_Wih"],
        inputs["dec_Whh"], inputs["dec_b"], inputs["cls_W"], inputs["cls_b"])
    in_maps = [
        host_prep_core(cfg, c, inputs["eout"], inputs["x_mask"], inputs["y"],
                       inputs["y_mask"], inputs["emb"], shared)
        for c in range(cfg.num_devices)
    ]
    res = run_bass_kernel_spmd(nc, in_maps,
                               core_ids=list(range(cfg.num_devices)),
                               trace=trace)
    out = host_post(cfg, [res.results[c]["logits"]
                          for c in range(cfg.num_devices)])
    return out, res


def kernel(**inputs):
    x_mask = np.asarray(inputs["x_mask"], np.float32)
    y_mask = np.asarray(inputs["y_mask"], np.float32)
    bound = float(np.abs(np.asarray(inputs["w_att_v"], np.float32)).sum())
    shift = max(0.0, bound - 60.0)
    cfg = Cfg(with_mbias=not bool((x_mask == 1.0).all()),
              with_ymask=not bool((y_mask == 1.0).all()),
              exp_shift=shift)
    out, _ = run(cfg, inputs)
    return out
